# revision 19
# baseline (speedup 1.0000x reference)
"""Trainium2 Bass kernel for gumbel-masked sparse attention.

Problem (hardcoded shapes): B=8, C=512, H=W=32 (N=1024), heads=8, hd=64, R=4.

    mq/mk  = (argmax over R of conv1x1(x, w*_s) + gumbel(u), axis=1) == 0
    q/k/v  = conv1x1(x, W*, b*)
    attn   = softmax over selected keys of (q^T k) * hd^-0.5
    out    = where(mq, attn @ v, v);  y = conv1x1(out, Wp, bp)

Distribution: data-parallel over batch B across the 8 NeuronCores (one
batch element per core), weights replicated.  The gumbel argmax masks are
computed on host (they must match the reference's fp32 CPU semantics
bit-for-bit — a single flipped mask position discretely changes a whole
output column), and the device kernel exploits the ~1/4 sparsity:
attention runs only on the selected (gathered, padded-to-NSEL) query/key
positions.

Formulation (avoids the baseline's full-N Wv@x pass):
    y = (Wp Wv) @ x + (Wp bv + bp)  +  A2^T @ E_neg
    A2^T[j, :] = (WpWv @ xq + Wp @ (-on) + Wp bv)[:, j]^T
    E_neg[j, iq[j]] = -1   (for j < count(mq))
so the correction  Wp@on - WpWv@xq - Wp@bv  lands exactly on the selected
query columns.  WpWv, Wp@bv and bp2 = bp + Wp@bv are precomputed on host.

Perf notes (from ntff traces of the previous version): each dma_start
costs ~600ns of *issue* time on the Sync engine regardless of size, so
all inputs are coalesced into 9 large first-use-ordered DMAs; the
per-pair softmax 1/Z partition broadcast is a 2-contraction matmul
instead of a DRAM bounce (removes 12 DMAs from the critical path); the
WpWv@x main-pass groups are emitted between attention pairs as PE filler.
"""

import numpy as np
import ml_dtypes

import concourse.bacc as bacc
import concourse.mybir as mybir
import concourse.tile as tile
from concourse.bass_utils import run_bass_kernel_spmd

BF16 = ml_dtypes.bfloat16
F32 = mybir.dt.float32
BF = mybir.dt.bfloat16

B, C, H, W = 8, 512, 32, 32
N = H * W                      # 1024
HEADS, HD = 8, 64
SCALE = HD ** -0.5             # 0.125
EPS = 1e-10
NEG = -30000.0                 # additive key-mask bias; exp(NEG + x) == 0
P = 128
CT = C // P                    # 4 channel tiles
NCH = N // 512                 # 2 free-dim chunks of the full N

NSEL_DEFAULT = 288             # padded selected-position count (max count 277)

TRACE = False                  # set True from test harness to profile
LAST_RESULT = None             # BassKernelResults of the last run (for tests)

_PROGRAM_CACHE = {}


# Drop the second all-engine barrier of TileContext's exit sequence
# (drain -> barrier -> sem clears -> barrier).  The gpsimd sem-clear stream
# still completes before the NEFF finishes (every engine stream must end),
# and no instruction follows it, so the final cross-engine alignment only
# adds ~3-4us of EVSEM butterfly to every execution.
def _slim_drain_and_barrier(self, tick_clock, wait_clock):
    from concourse.vector_clock import ScopedClock

    drain_inst = self.nc.sync.drain()
    wait_clock.add_sem_waits(
        drain_inst.ins, ScopedClock({None: tick_clock.global_clock})
    )
    self.nc.all_engine_barrier()
    popped = self.nc._tile_sem_poison_stack.pop()
    assert popped is self._sem_poison
    self.nc.clear_and_free_semaphores(list(self.sems.allocated().values()))


tile.TileContext._drain_and_barrier = _slim_drain_and_barrier


def _chunks(total, step=P):
    return [(o, min(step, total - o)) for o in range(0, total, step)]


def _build_program(NSEL, zbias):
    MT = len(_chunks(NSEL))    # m-chunks over selected keys
    JT = MT                    # j-tiles over selected queries
    JTP = JT * P               # emat row padding
    WQX = C + NSEL             # width of the wq|xq (wk|xk) bundles
    NC_ = 8 + CT + MT          # consts width
    nc = bacc.Bacc("TRN2", target_bir_lowering=False, debug=False, num_devices=8)

    # consts layout [P, NC_] f32:
    #   cols 0:4 bq, 4:8 bk, 8:12 bp2 (=bp + Wp@bv), 12:12+MT kbias
    consts_e = nc.declare_dram_parameter("consts", [P, NC_], F32, isOutput=False)
    rows2_e = nc.declare_dram_parameter("rows2", [1, 2 * C], BF, isOutput=False)
    wqxq_e = nc.declare_dram_parameter("wqxq", [C, WQX], BF, isOutput=False)
    wkxk_e = nc.declare_dram_parameter("wkxk", [C, WQX], BF, isOutput=False)
    wv_e = nc.declare_dram_parameter("wvT", [C, C], BF, isOutput=False)
    x_e = nc.declare_dram_parameter("xbf", [C, N], BF, isOutput=False)
    wpwv_e = nc.declare_dram_parameter("wpwvT", [C, C], BF, isOutput=False)
    wp_e = nc.declare_dram_parameter("wpT", [C, C], BF, isOutput=False)
    e_e = nc.declare_dram_parameter("emat", [JTP, N], BF, isOutput=False)
    y_e = nc.declare_dram_parameter("y", [C, N], BF, isOutput=True)

    def folded(ap):
        # [t*128, w] dram -> [128, t, w] (lands in sbuf as chunk kc at cols kc*w)
        return ap[:].rearrange("(t p) n -> p t n", p=P)

    def unfold(sb_tile, t):
        # [128, t*w] sbuf tile viewed [128, t, w] to match folded(dram)
        return sb_tile[:].rearrange("p (t n) -> p t n", t=t)

    with tile.TileContext(nc) as tc:
        with (
            tc.tile_pool(name="sb", bufs=1) as sb,
            tc.tile_pool(name="psA", bufs=4, space="PSUM") as psA,
            tc.tile_pool(name="psB", bufs=2, space="PSUM") as psB,
            tc.tile_pool(name="psC", bufs=2, space="PSUM") as psC,
        ):
            def sbt(tag, shape, dtype=BF):
                return sb.tile(shape, dtype, name=tag, tag=tag)

            def cpst(w=512):
                return psC.tile([P, w], F32, name="mm", tag="mm")

            # ---- coalesced input DMAs, first-use order; the two big
            # critical bundles go first on the sync HWDGE queue, the small
            # consts ride the scalar HWDGE queue in parallel ----
            # q/k bundles are split: the gathered data + tile-0 weight
            # columns stream first so pair 0's projections start ~4us
            # earlier; the remaining weight tiles follow wv.
            HEADW = NSEL + P
            wqxq = sbt("wqxq", [P, CT * WQX])
            wkxk = sbt("wkxk", [P, CT * WQX])
            nc.sync.dma_start(out=unfold(wqxq, CT)[:, :, 0:HEADW],
                              in_=folded(wqxq_e)[:, :, 0:HEADW])
            nc.sync.dma_start(out=unfold(wkxk, CT)[:, :, 0:HEADW],
                              in_=folded(wkxk_e)[:, :, 0:HEADW])
            consts = sbt("consts", [P, NC_], F32)
            nc.scalar.dma_start(out=consts[:], in_=consts_e[:])
            rows2 = sbt("rows2", [1, 2 * C])
            nc.scalar.dma_start(out=rows2[:], in_=rows2_e[:])
            wv = sbt("wv", [P, CT * C])
            nc.sync.dma_start(out=unfold(wv, CT), in_=folded(wv_e))
            nc.sync.dma_start(out=unfold(wqxq, CT)[:, :, HEADW:WQX],
                              in_=folded(wqxq_e)[:, :, HEADW:WQX])
            nc.sync.dma_start(out=unfold(wkxk, CT)[:, :, HEADW:WQX],
                              in_=folded(wkxk_e)[:, :, HEADW:WQX])
            x = sbt("x", [P, CT * N])
            nc.sync.dma_start(out=unfold(x, CT), in_=folded(x_e))
            wpwv = sbt("wpwv", [P, CT * C])
            nc.sync.dma_start(out=unfold(wpwv, CT), in_=folded(wpwv_e))
            wp = sbt("wp", [P, CT * C])
            nc.sync.dma_start(out=unfold(wp, CT), in_=folded(wp_e))
            emat = sbt("emat", [P, JT * N])
            nc.sync.dma_start(out=unfold(emat, JT), in_=folded(e_e))

            bq = consts[:, 0:CT]
            bk = consts[:, CT:2 * CT]
            bp2 = consts[:, 2 * CT:3 * CT]
            kb = consts[:, 3 * CT:3 * CT + MT]
            bvrow = rows2[0:1, 0:C]
            wpbv = rows2[0:1, C:2 * C]

            def xq_c(kc):
                return wqxq[:, kc * WQX:kc * WQX + NSEL]

            def wq_c(kc):
                return wqxq[:, kc * WQX + NSEL:(kc + 1) * WQX]

            def xk_c(kc):
                return wkxk[:, kc * WQX:kc * WQX + NSEL]

            def wk_c(kc):
                return wkxk[:, kc * WQX + NSEL:(kc + 1) * WQX]

            def wv_c(kc):
                return wv[:, kc * C:(kc + 1) * C]

            def wpwv_c(kc):
                return wpwv[:, kc * C:(kc + 1) * C]

            def wp_c(kc):
                return wp[:, kc * C:(kc + 1) * C]

            ones1 = sbt("ones1", [1, P])
            nc.vector.memset(ones1[:], 1.0)

            # dummy activation with no data deps: pulls the ACT_TABLE_LOAD
            # (~1.3us) to t=0 instead of serializing before the first real exp
            warm = sbt("warm", [1, 1], F32)
            nc.vector.memset(warm[:], 1.0)
            nc.scalar.activation(warm[:], warm[:], mybir.ActivationFunctionType.Exp)

            # dummy matmuls while the input DMAs land: sustained PE activity
            # flips the HAM clock gate to 2.4 GHz before real work
            wmm = sbt("wmm", [P, 512])
            nc.vector.memset(wmm[:], 0.0)
            for _ in range(5):
                wps = psC.tile([P, 512], F32, name="wps", tag="mm")
                nc.tensor.matmul(wps[:], wmm[:, :P], wmm[:], start=True, stop=True)

            # ---- q/k projections: [C, NSEL] bf16; tile t+1 is emitted
            # after pair t's attention (weights for tiles 1-3 stream late)
            q_sb = sbt("q", [P, CT * NSEL])
            k_sb = sbt("k", [P, CT * NSEL])

            def emit_proj(t):
                for wgt, rhs, bias, out in (
                    (wq_c, xq_c, bq, q_sb), (wk_c, xk_c, bk, k_sb),
                ):
                    psm = cpst(NSEL)
                    for kc in range(CT):
                        nc.tensor.matmul(
                            psm[:],
                            wgt(kc)[:, t * P:(t + 1) * P],
                            rhs(kc),
                            start=(kc == 0), stop=(kc == CT - 1),
                        )
                    nc.vector.tensor_scalar_add(
                        out[:, t * NSEL:(t + 1) * NSEL], psm[:], bias[:, t:t + 1],
                    )

            emit_proj(0)

            # ---- vT_sel[m, 65h + d] = v_sel[64h + d, m]; column 65h+64 = 1.0
            # (ones column makes the PV matmul also produce Z = sum_m S[m, j])
            # Emitted inside pair 0's QK stream: pair 0 has no other PE
            # filler while ACT works through its exps.
            vt_sb = [sbt(f"vt{mt}", [P, HEADS * (HD + 1)]) for mt in range(MT)]

            def emit_vt(mt):
                mo, mw = _chunks(NSEL)[mt]
                psm = cpst(C)
                for kc in range(CT):
                    nc.tensor.matmul(
                        psm[0:mw, :],
                        xk_c(kc)[:, mo:mo + mw],
                        wv_c(kc),
                        start=(kc == 0), stop=(zbias and kc == CT - 1),
                    )
                if not zbias:
                    nc.tensor.matmul(psm[0:mw, :], ones1[:, 0:mw], bvrow,
                                     start=False, stop=True)
                vt_view = vt_sb[mt][:].rearrange("p (h d) -> p h d", d=HD + 1)
                nc.vector.tensor_copy(
                    vt_view[0:mw, :, 0:HD],
                    psm[0:mw, :].rearrange("p (h d) -> p h d", d=HD),
                )
                nc.vector.memset(vt_view[0:mw, :, HD:HD + 1], 1.0)

            # ---- WpWv@x main-pass groups (PE filler between pairs) ----
            y_sb = sbt("y", [P, CT * N])
            ymain = [(co, nch) for co in range(CT) for nch in range(NCH)]

            def emit_ymain(gi):
                co, nch = ymain[gi]
                psm = psA.tile([P, 512], F32, name="ym", tag="qk")
                for kc in range(CT):
                    nc.tensor.matmul(
                        psm[:],
                        wpwv_c(kc)[:, co * P:(co + 1) * P],
                        x[:, kc * N + nch * 512:kc * N + (nch + 1) * 512],
                        start=(kc == 0), stop=(kc == CT - 1),
                    )
                seg = y_sb[:, co * N + nch * 512:co * N + (nch + 1) * 512]
                nc.vector.tensor_scalar_add(seg, psm[:], bp2[:, co:co + 1])

            # ---- attention (selected keys m in partitions, queries j free) ----
            # S[m, j] = exp(scale * k_m . q_j + kbias[m]), bf16.  Per pair:
            # Z sits in po row 64 (vt ones column); two DVE row-copies pull it
            # to SBUF, a ones-matmul broadcasts it across the head partitions,
            # reciprocal_approx_fast (DVE, ~18 bits) gives alpha with no ACT
            # work, and on_neg = (alpha * -1) * po is fused in per-half STTs
            # reading the PV psum directly.
            po = [None] * HEADS
            on_sb = [sbt(f"on{t}", [P, NSEL]) for t in range(CT)]

            def emit_a2_head(jt, jw, jo, psm, upto):
                # A2 psum partial: optional Wp@bv bias + on-parts for pairs
                # < upto.  (No WpWv@xq term: the host zeroes the selected
                # query columns of x, so the main pass contributes nothing
                # at the scatter targets.)
                first = True
                if not zbias:
                    nc.tensor.matmul(psm[0:jw, :], ones1[:, 0:jw], wpbv,
                                     start=True, stop=False)
                    first = False
                for kc in range(upto):
                    nc.tensor.matmul(
                        psm[0:jw, :], on_sb[kc][:, jo:jo + jw], wp_c(kc),
                        start=first and kc == 0, stop=False,
                    )

            at_sb = [sbt(f"at{j}", [P, C]) for j in range(JT)]
            a2ps = [None] * JT
            ynext = 0
            for t in range(CT):  # head pair (2t, 2t+1)
                if t == CT - 1:
                    # last filler groups go BEFORE the final pair so the
                    # tail-critical alpha -> A2 chain is not queued behind
                    # them on the in-order PE
                    while ynext < len(ymain):
                        emit_ymain(ynext)
                        ynext += 1
                for half in range(2):
                    h = 2 * t + half
                    po[h] = psB.tile([HD + 1, NSEL], F32, name="pv", tag="pv")
                s_pairs = []
                for mj, (mo, mw) in enumerate(_chunks(NSEL)):
                    # the two QK matmuls run CONCURRENTLY on the PE via
                    # tile_position row-tiling (contraction is only 64 wide)
                    qkps = []
                    for half in range(2):
                        psm = psA.tile([P, NSEL], F32, name="qk", tag="qk")
                        nc.tensor.matmul(
                            psm[0:mw, :],
                            k_sb[half * HD:(half + 1) * HD,
                                 t * NSEL + mo:t * NSEL + mo + mw],
                            q_sb[half * HD:(half + 1) * HD,
                                 t * NSEL:(t + 1) * NSEL],
                            start=True, stop=True,
                            tile_position=(half * HD, 0),
                        )
                        qkps.append(psm)
                    s_pair = sbt(f"s{t}_{mj}", [P, 2 * NSEL])
                    s_pairs.append(s_pair)
                    for half in range(2):
                        nc.scalar.activation(
                            s_pair[0:mw, half * NSEL:(half + 1) * NSEL],
                            qkps[half][0:mw, :],
                            mybir.ActivationFunctionType.Exp,
                            bias=kb[0:mw, mj:mj + 1], scale=SCALE,
                        )
                    if t == 0:
                        # pair 0 has no projections/filler to absorb the
                        # exp latency; vt construction fills the PE instead
                        emit_vt(mj)
                    else:
                        for half in range(2):
                            h = 2 * t + half
                            nc.tensor.matmul(
                                po[h][:],
                                vt_sb[mj][0:mw, h * (HD + 1):(h + 1) * (HD + 1)],
                                s_pair[0:mw, half * NSEL:(half + 1) * NSEL],
                                start=(mj == 0), stop=(mj == MT - 1),
                            )
                if t == 0:
                    for mj, (mo, mw) in enumerate(_chunks(NSEL)):
                        for half in range(2):
                            h = 2 * t + half
                            nc.tensor.matmul(
                                po[h][:],
                                vt_sb[mj][0:mw, h * (HD + 1):(h + 1) * (HD + 1)],
                                s_pairs[mj][0:mw, half * NSEL:(half + 1) * NSEL],
                                start=(mj == 0), stop=(mj == MT - 1),
                            )
                # per-pair 1/Z chain (overlaps the next pair's attention).
                # The next tile's projections / filler / A2 heads sit between
                # PV and the zbc broadcast in the PE stream so the PE is
                # never waiting on the DVE-side z2 row copies.
                z2 = sbt(f"z{t}", [1, 2 * NSEL])
                for half in range(2):
                    h = 2 * t + half
                    nc.vector.tensor_copy(
                        z2[0:1, half * NSEL:(half + 1) * NSEL],
                        po[h][HD:HD + 1, :],
                    )
                if t + 1 < CT:
                    emit_proj(t + 1)
                if t == 1:
                    for _ in range(2):
                        emit_ymain(ynext)
                        ynext += 1
                if t == 2:
                    for _ in range(4):
                        emit_ymain(ynext)
                        ynext += 1
                if t == CT - 1:
                    # A2 heads for j-tiles 0/1 (pairs 0-2 on-parts) cover the
                    # z2 -> zbc -> recip -> STT latency of the last pair
                    for jt in (0, 1):
                        jo, jw = _chunks(NSEL)[jt]
                        a2ps[jt] = cpst(C)
                        emit_a2_head(jt, jw, jo, a2ps[jt], upto=CT - 1)
                zbc = psA.tile([P, NSEL], F32, name="zbc", tag="qk")
                for half in range(2):
                    nc.tensor.matmul(
                        zbc[half * HD:(half + 1) * HD, :],
                        ones1[:, 0:HD],
                        z2[0:1, half * NSEL:(half + 1) * NSEL],
                        start=True, stop=True,
                    )
                alpbc = sbt(f"alp{t}", [P, NSEL], F32)
                nc.vector.reciprocal_approx_fast(out=alpbc[:], in_=zbc[:])
                for half in range(2):
                    h = 2 * t + half
                    nc.vector.scalar_tensor_tensor(
                        on_sb[t][half * HD:(half + 1) * HD, :],
                        alpbc[half * HD:(half + 1) * HD, :], -1.0,
                        po[h][0:HD, :],
                        op0=mybir.AluOpType.mult, op1=mybir.AluOpType.mult,
                    )

            # ---- A2^T[j, :] = (WpWv@xq + Wp@(-on) + Wp@bv)^T ----
            # j-tiles 0/1 only need the pair-3 on chunk + eviction; tile 2
            # runs in full.  Evictions ride ScalarE (idle after the last exp).
            for jt, (jo, jw) in enumerate(_chunks(NSEL)):
                if a2ps[jt] is not None:
                    psm = a2ps[jt]
                    nc.tensor.matmul(
                        psm[0:jw, :], on_sb[CT - 1][:, jo:jo + jw],
                        wp_c(CT - 1), start=False, stop=True,
                    )
                else:
                    psm = cpst(C)
                    emit_a2_head(jt, jw, jo, psm, upto=CT - 1)
                    nc.tensor.matmul(
                        psm[0:jw, :], on_sb[CT - 1][:, jo:jo + jw],
                        wp_c(CT - 1), start=False, stop=True,
                    )
                if jt % 2 == 0:
                    nc.vector.tensor_copy(at_sb[jt][0:jw, :], psm[0:jw, :])
                else:
                    nc.scalar.activation(
                        at_sb[jt][0:jw, :], psm[0:jw, :],
                        mybir.ActivationFunctionType.Identity,
                    )

            # ---- y += A2^T^T @ E_neg (column scatter of the correction) ----
            for co in range(CT):
                for nch in range(NCH):
                    psm = psB.tile([P, 512], F32, name="ye", tag="pv")
                    for jt, (jo, jw) in enumerate(_chunks(NSEL)):
                        nc.tensor.matmul(
                            psm[:],
                            at_sb[jt][0:jw, co * P:(co + 1) * P],
                            emat[0:jw, jt * N + nch * 512:jt * N + (nch + 1) * 512],
                            start=(jt == 0), stop=(jt == JT - 1),
                        )
                    seg = y_sb[:, co * N + nch * 512:co * N + (nch + 1) * 512]
                    nc.vector.tensor_tensor(
                        seg, seg, psm[:], op=mybir.AluOpType.add,
                    )
                    nc.sync.dma_start(
                        out=y_e[:]
                        .rearrange("(t p) n -> t p n", p=P)[co]
                        [:, nch * 512:(nch + 1) * 512],
                        in_=seg,
                    )

    # The greedy ACT-table-load pass alternates between exp-only and ln-only
    # table sets for our Exp/Ln/Identity/Copy mix, inserting ~9 ACT_TABLE_LOADs
    # (~1.3us each).  natural_log_exp_and_others contains all four functions;
    # make it the only candidate (list positions must stay aligned with
    # act_info.json indices, so empty the competitors instead of removing).
    import concourse.bacc as bacc_mod

    WANT = "natural_log_exp_and_others"
    orig_tables = bacc_mod.get_activation_tables

    def one_set_tables(arch):
        tabs = orig_tables(arch)
        ours = {
            mybir.ActivationFunctionType.Exp,
            mybir.ActivationFunctionType.Ln,
            mybir.ActivationFunctionType.Identity,
            mybir.ActivationFunctionType.Copy,
        }
        return {
            name: (fns if name == WANT else fns - ours)
            for name, fns in tabs.items()
        }

    bacc_mod.get_activation_tables = one_set_tables
    try:
        nc.compile()
    finally:
        bacc_mod.get_activation_tables = orig_tables
    return nc


def _get_program(NSEL, zbias):
    key = (NSEL, zbias)
    if key not in _PROGRAM_CACHE:
        _PROGRAM_CACHE[key] = _build_program(NSEL, zbias)
    return _PROGRAM_CACHE[key]


def _sel_masks(x, u, ws, bs):
    """Bit-exact replica of the reference's gumbel argmax mask (fp32, CPU jax)."""
    import jax
    import jax.numpy as jnp

    cpu = jax.devices("cpu")[0]
    with jax.default_device(cpu):
        xj = jax.device_put(jnp.asarray(x, jnp.float32), cpu)
        uj = jax.device_put(jnp.asarray(u, jnp.float32), cpu)
        wj = jax.device_put(jnp.asarray(ws, jnp.float32), cpu)
        bj = jax.device_put(jnp.asarray(bs, jnp.float32), cpu)
        logits = jnp.einsum("bchw,oc->bohw", xj, wj) + bj[None, :, None, None]
        g = -jnp.log(-jnp.log(uj + EPS) + EPS)
        m = jnp.argmax(logits + g, axis=1) == 0
        return np.asarray(m).reshape(x.shape[0], N)


def _col_layout(vec, nt):
    """[nt*128] -> [128, nt] with column t = vec[128t:128(t+1)]."""
    return np.ascontiguousarray(vec.reshape(nt, P).T)


def kernel(x, u_q, u_k, wq_s, bq_s, wk_s, bk_s, Wq, bq, Wk, bk, Wv, bv, Wp, bp):
    global LAST_RESULT
    x = np.asarray(x, np.float32)
    u_q, u_k = np.asarray(u_q, np.float32), np.asarray(u_k, np.float32)

    mq = _sel_masks(x, u_q, np.asarray(wq_s), np.asarray(bq_s))
    mk = _sel_masks(x, u_k, np.asarray(wk_s), np.asarray(bk_s))

    idx_q = [np.nonzero(mq[b])[0] for b in range(B)]
    idx_k = [np.nonzero(mk[b])[0] for b in range(B)]
    max_cnt = max(max(len(i) for i in idx_q), max(len(i) for i in idx_k))
    NSEL = NSEL_DEFAULT
    while NSEL < max_cnt:
        NSEL += 32
    MT = len(_chunks(NSEL))
    JTP = MT * P

    Wq32, Wk32 = np.asarray(Wq, np.float32), np.asarray(Wk, np.float32)
    Wv32, Wp32 = np.asarray(Wv, np.float32), np.asarray(Wp, np.float32)
    bq32, bk32 = np.asarray(bq, np.float32), np.asarray(bk, np.float32)
    bv32, bp32 = np.asarray(bv, np.float32), np.asarray(bp, np.float32)

    WpWv32 = Wp32 @ Wv32
    wpbv32 = Wp32 @ bv32
    bp2 = bp32 + wpbv32

    wqT = np.ascontiguousarray(Wq32.T)
    wkT = np.ascontiguousarray(Wk32.T)
    wvT = np.ascontiguousarray(Wv32.T).astype(BF16)
    wpT = np.ascontiguousarray(Wp32.T).astype(BF16)
    wpwvT = np.ascontiguousarray(WpWv32.T).astype(BF16)
    rows2 = np.concatenate([bv32, wpbv32]).reshape(1, 2 * C).astype(BF16)

    # consts [P, 8 + CT + MT]
    kbias_cols = MT

    xf = x.reshape(B, C, N)
    in_maps = []
    for b in range(B):
        iq, ik = idx_q[b], idx_k[b]
        cq, ck = len(iq), len(ik)
        iq_pad = np.pad(iq, (0, NSEL - cq))
        ik_pad = np.pad(ik, (0, NSEL - ck))

        kbias = np.zeros(MT * P, np.float32)
        kbias[ck:NSEL] = NEG
        kbias[NSEL:] = NEG
        consts = np.concatenate([
            _col_layout(bq32, CT), _col_layout(bk32, CT),
            _col_layout(bp2, CT),
            _col_layout(kbias, MT)[:, :kbias_cols],
        ], axis=1).astype(np.float32)
        consts = np.ascontiguousarray(consts)

        emat = np.zeros((JTP, N), BF16)
        emat[np.arange(cq), iq[:cq]] = -1.0

        xb = xf[b].copy()
        xb[:, iq] = 0.0            # main pass must not contribute at scatter targets
        xb = xb.astype(BF16)
        xqg = np.ascontiguousarray(xf[b][:, iq_pad]).astype(BF16)
        xkg = np.ascontiguousarray(xf[b][:, ik_pad]).astype(BF16)
        wqxq = np.ascontiguousarray(
            np.concatenate([xf[b][:, iq_pad], wqT], axis=1)).astype(BF16)
        wkxk = np.ascontiguousarray(
            np.concatenate([xf[b][:, ik_pad], wkT], axis=1)).astype(BF16)
        del xqg, xkg

        in_maps.append({
            "consts": consts,
            "rows2": rows2,
            "wqxq": wqxq,
            "wkxk": wkxk,
            "wvT": wvT,
            "xbf": xb,
            "wpwvT": wpwvT,
            "wpT": wpT,
            "emat": emat,
        })

    zbias = not (bq32.any() or bk32.any() or bv32.any() or bp32.any())
    nc = _get_program(NSEL, zbias)
    res = run_bass_kernel_spmd(nc, in_maps, list(range(B)), trace=TRACE)
    LAST_RESULT = res

    y = np.stack([np.asarray(res.results[b]["y"], np.float32) for b in range(B)])
    return y.reshape(B, C, H, W)


# revision 20
# speedup vs baseline: 1.0217x; 1.0217x over previous
"""Trainium2 Bass kernel for gumbel-masked sparse attention.

Problem (hardcoded shapes): B=8, C=512, H=W=32 (N=1024), heads=8, hd=64, R=4.

    mq/mk  = (argmax over R of conv1x1(x, w*_s) + gumbel(u), axis=1) == 0
    q/k/v  = conv1x1(x, W*, b*)
    attn   = softmax over selected keys of (q^T k) * hd^-0.5
    out    = where(mq, attn @ v, v);  y = conv1x1(out, Wp, bp)

Distribution: data-parallel over batch B across the 8 NeuronCores (one
batch element per core), weights replicated.  The gumbel argmax masks are
computed on host (they must match the reference's fp32 CPU semantics
bit-for-bit — a single flipped mask position discretely changes a whole
output column), and the device kernel exploits the ~1/4 sparsity:
attention runs only on the selected (gathered, padded-to-NSEL) query/key
positions.

Formulation (avoids the baseline's full-N Wv@x pass):
    y = (Wp Wv) @ x + (Wp bv + bp)  +  A2^T @ E_neg
    A2^T[j, :] = (WpWv @ xq + Wp @ (-on) + Wp bv)[:, j]^T
    E_neg[j, iq[j]] = -1   (for j < count(mq))
so the correction  Wp@on - WpWv@xq - Wp@bv  lands exactly on the selected
query columns.  WpWv, Wp@bv and bp2 = bp + Wp@bv are precomputed on host.

Perf notes (from ntff traces of the previous version): each dma_start
costs ~600ns of *issue* time on the Sync engine regardless of size, so
all inputs are coalesced into 9 large first-use-ordered DMAs; the
per-pair softmax 1/Z partition broadcast is a 2-contraction matmul
instead of a DRAM bounce (removes 12 DMAs from the critical path); the
WpWv@x main-pass groups are emitted between attention pairs as PE filler.
"""

import numpy as np
import ml_dtypes

import concourse.bacc as bacc
import concourse.mybir as mybir
import concourse.tile as tile
from concourse.bass_utils import run_bass_kernel_spmd

BF16 = ml_dtypes.bfloat16
F32 = mybir.dt.float32
BF = mybir.dt.bfloat16

B, C, H, W = 8, 512, 32, 32
N = H * W                      # 1024
HEADS, HD = 8, 64
SCALE = HD ** -0.5             # 0.125
EPS = 1e-10
NEG = -30000.0                 # additive key-mask bias; exp(NEG + x) == 0
P = 128
CT = C // P                    # 4 channel tiles
NCH = N // 512                 # 2 free-dim chunks of the full N

NSEL_DEFAULT = 288             # padded selected-position count (max count 277)

TRACE = False                  # set True from test harness to profile
LAST_RESULT = None             # BassKernelResults of the last run (for tests)

_PROGRAM_CACHE = {}


# Drop the second all-engine barrier of TileContext's exit sequence
# (drain -> barrier -> sem clears -> barrier).  The gpsimd sem-clear stream
# still completes before the NEFF finishes (every engine stream must end),
# and no instruction follows it, so the final cross-engine alignment only
# adds ~3-4us of EVSEM butterfly to every execution.
def _slim_drain_and_barrier(self, tick_clock, wait_clock):
    from concourse.vector_clock import ScopedClock

    drain_inst = self.nc.sync.drain()
    wait_clock.add_sem_waits(
        drain_inst.ins, ScopedClock({None: tick_clock.global_clock})
    )
    self.nc.all_engine_barrier()
    popped = self.nc._tile_sem_poison_stack.pop()
    assert popped is self._sem_poison
    self.nc.clear_and_free_semaphores(list(self.sems.allocated().values()))


tile.TileContext._drain_and_barrier = _slim_drain_and_barrier


def _chunks(total, step=P):
    return [(o, min(step, total - o)) for o in range(0, total, step)]


def _build_program(NSEL, zbias):
    MT = len(_chunks(NSEL))    # m-chunks over selected keys
    JT = MT                    # j-tiles over selected queries
    JTP = JT * P               # emat row padding
    WQX = C + NSEL             # width of the wq|xq (wk|xk) bundles
    NC_ = 8 + CT + MT          # consts width
    nc = bacc.Bacc("TRN2", target_bir_lowering=False, debug=False, num_devices=8)

    # consts layout [P, NC_] f32:
    #   cols 0:4 bq, 4:8 bk, 8:12 bp2 (=bp + Wp@bv), 12:12+MT kbias
    consts_e = nc.declare_dram_parameter("consts", [P, NC_], F32, isOutput=False)
    rows2_e = nc.declare_dram_parameter("rows2", [1, 2 * C], BF, isOutput=False)
    wqxq_e = nc.declare_dram_parameter("wqxq", [C, WQX], BF, isOutput=False)
    wkxk_e = nc.declare_dram_parameter("wkxk", [C, WQX], BF, isOutput=False)
    wv_e = nc.declare_dram_parameter("wvT", [C, C], BF, isOutput=False)
    x_e = nc.declare_dram_parameter("xbf", [C, N], BF, isOutput=False)
    wpwv_e = nc.declare_dram_parameter("wpwvT", [C, C], BF, isOutput=False)
    wp_e = nc.declare_dram_parameter("wpT", [C, C], BF, isOutput=False)
    e_e = nc.declare_dram_parameter("emat", [JTP, N], BF, isOutput=False)
    y_e = nc.declare_dram_parameter("y", [C, N], BF, isOutput=True)

    def folded(ap):
        # [t*128, w] dram -> [128, t, w] (lands in sbuf as chunk kc at cols kc*w)
        return ap[:].rearrange("(t p) n -> p t n", p=P)

    def unfold(sb_tile, t):
        # [128, t*w] sbuf tile viewed [128, t, w] to match folded(dram)
        return sb_tile[:].rearrange("p (t n) -> p t n", t=t)

    with tile.TileContext(nc) as tc:
        with (
            tc.tile_pool(name="sb", bufs=1) as sb,
            tc.tile_pool(name="psA", bufs=4, space="PSUM") as psA,
            tc.tile_pool(name="psB", bufs=2, space="PSUM") as psB,
            tc.tile_pool(name="psC", bufs=2, space="PSUM") as psC,
        ):
            def sbt(tag, shape, dtype=BF):
                return sb.tile(shape, dtype, name=tag, tag=tag)

            def cpst(w=512):
                return psC.tile([P, w], F32, name="mm", tag="mm")

            # ---- coalesced input DMAs, first-use order; the two big
            # critical bundles go first on the sync HWDGE queue, the small
            # consts ride the scalar HWDGE queue in parallel ----
            # q/k bundles are split: the gathered data + tile-0 weight
            # columns stream first so pair 0's projections start ~4us
            # earlier; the remaining weight tiles follow wv.
            HEADW = NSEL + P
            wqxq = sbt("wqxq", [P, CT * WQX])
            wkxk = sbt("wkxk", [P, CT * WQX])
            nc.sync.dma_start(out=unfold(wqxq, CT)[:, :, 0:HEADW],
                              in_=folded(wqxq_e)[:, :, 0:HEADW])
            nc.sync.dma_start(out=unfold(wkxk, CT)[:, :, 0:HEADW],
                              in_=folded(wkxk_e)[:, :, 0:HEADW])
            consts = sbt("consts", [P, NC_], F32)
            nc.scalar.dma_start(out=consts[:], in_=consts_e[:])
            rows2 = sbt("rows2", [1, 2 * C])
            nc.scalar.dma_start(out=rows2[:], in_=rows2_e[:])
            wv = sbt("wv", [P, CT * C])
            nc.sync.dma_start(out=unfold(wv, CT), in_=folded(wv_e))
            nc.sync.dma_start(out=unfold(wqxq, CT)[:, :, HEADW:WQX],
                              in_=folded(wqxq_e)[:, :, HEADW:WQX])
            nc.sync.dma_start(out=unfold(wkxk, CT)[:, :, HEADW:WQX],
                              in_=folded(wkxk_e)[:, :, HEADW:WQX])
            x = sbt("x", [P, CT * N])
            nc.sync.dma_start(out=unfold(x, CT), in_=folded(x_e))
            wpwv = sbt("wpwv", [P, CT * C])
            nc.sync.dma_start(out=unfold(wpwv, CT), in_=folded(wpwv_e))
            wp = sbt("wp", [P, CT * C])
            nc.sync.dma_start(out=unfold(wp, CT), in_=folded(wp_e))
            emat = sbt("emat", [P, JT * N])
            nc.sync.dma_start(out=unfold(emat, JT), in_=folded(e_e))

            bq = consts[:, 0:CT]
            bk = consts[:, CT:2 * CT]
            bp2 = consts[:, 2 * CT:3 * CT]
            kb = consts[:, 3 * CT:3 * CT + MT]
            bvrow = rows2[0:1, 0:C]
            wpbv = rows2[0:1, C:2 * C]

            def xq_c(kc):
                return wqxq[:, kc * WQX:kc * WQX + NSEL]

            def wq_c(kc):
                return wqxq[:, kc * WQX + NSEL:(kc + 1) * WQX]

            def xk_c(kc):
                return wkxk[:, kc * WQX:kc * WQX + NSEL]

            def wk_c(kc):
                return wkxk[:, kc * WQX + NSEL:(kc + 1) * WQX]

            def wv_c(kc):
                return wv[:, kc * C:(kc + 1) * C]

            def wpwv_c(kc):
                return wpwv[:, kc * C:(kc + 1) * C]

            def wp_c(kc):
                return wp[:, kc * C:(kc + 1) * C]

            ones1 = sbt("ones1", [1, P])
            nc.vector.memset(ones1[:], 1.0)

            # dummy activation with no data deps: pulls the ACT_TABLE_LOAD
            # (~1.3us) to t=0 instead of serializing before the first real exp
            warm = sbt("warm", [1, 1], F32)
            nc.vector.memset(warm[:], 1.0)
            nc.scalar.activation(warm[:], warm[:], mybir.ActivationFunctionType.Exp)

            # dummy matmuls while the input DMAs land: sustained PE activity
            # flips the HAM clock gate to 2.4 GHz before real work
            wmm = sbt("wmm", [P, 512])
            nc.vector.memset(wmm[:], 0.0)
            for _ in range(5):
                wps = psC.tile([P, 512], F32, name="wps", tag="mm")
                nc.tensor.matmul(wps[:], wmm[:, :P], wmm[:], start=True, stop=True)

            # ---- q/k projections: [C, NSEL] bf16; tile t+1 is emitted
            # after pair t's attention (weights for tiles 1-3 stream late)
            q_sb = sbt("q", [P, CT * NSEL])
            k_sb = sbt("k", [P, CT * NSEL])

            def emit_proj(t):
                for wgt, rhs, bias, out in (
                    (wq_c, xq_c, bq, q_sb), (wk_c, xk_c, bk, k_sb),
                ):
                    psm = cpst(NSEL)
                    for kc in range(CT):
                        nc.tensor.matmul(
                            psm[:],
                            wgt(kc)[:, t * P:(t + 1) * P],
                            rhs(kc),
                            start=(kc == 0), stop=(kc == CT - 1),
                        )
                    nc.vector.tensor_scalar_add(
                        out[:, t * NSEL:(t + 1) * NSEL], psm[:], bias[:, t:t + 1],
                    )

            emit_proj(0)

            # ---- vT_sel[m, 65h + d] = v_sel[64h + d, m]; column 65h+64 = 1.0
            # (ones column makes the PV matmul also produce Z = sum_m S[m, j])
            # Emitted inside pair 0's QK stream: pair 0 has no other PE
            # filler while ACT works through its exps.
            vt_sb = [sbt(f"vt{mt}", [P, HEADS * (HD + 1)]) for mt in range(MT)]

            def emit_vt(mt):
                mo, mw = _chunks(NSEL)[mt]
                psm = cpst(C)
                for kc in range(CT):
                    nc.tensor.matmul(
                        psm[0:mw, :],
                        xk_c(kc)[:, mo:mo + mw],
                        wv_c(kc),
                        start=(kc == 0), stop=(zbias and kc == CT - 1),
                    )
                if not zbias:
                    nc.tensor.matmul(psm[0:mw, :], ones1[:, 0:mw], bvrow,
                                     start=False, stop=True)
                vt_view = vt_sb[mt][:].rearrange("p (h d) -> p h d", d=HD + 1)
                nc.vector.tensor_copy(
                    vt_view[0:mw, :, 0:HD],
                    psm[0:mw, :].rearrange("p (h d) -> p h d", d=HD),
                )
                nc.vector.memset(vt_view[0:mw, :, HD:HD + 1], 1.0)

            # ---- WpWv@x main-pass groups (PE filler between pairs) ----
            y_sb = sbt("y", [P, CT * N])
            ymain = [(co, nch) for co in range(CT) for nch in range(NCH)]

            def emit_ymain(gi):
                co, nch = ymain[gi]
                psm = psA.tile([P, 512], F32, name="ym", tag="qk")
                for kc in range(CT):
                    nc.tensor.matmul(
                        psm[:],
                        wpwv_c(kc)[:, co * P:(co + 1) * P],
                        x[:, kc * N + nch * 512:kc * N + (nch + 1) * 512],
                        start=(kc == 0), stop=(kc == CT - 1),
                    )
                seg = y_sb[:, co * N + nch * 512:co * N + (nch + 1) * 512]
                nc.vector.tensor_scalar_add(seg, psm[:], bp2[:, co:co + 1])

            # ---- attention (selected keys m in partitions, queries j free) ----
            # S[m, j] = exp(scale * k_m . q_j + kbias[m]), bf16.  Per pair:
            # Z sits in po row 64 (vt ones column); two DVE row-copies pull it
            # to SBUF, a ones-matmul broadcasts it across the head partitions,
            # reciprocal_approx_fast (DVE, ~18 bits) gives alpha with no ACT
            # work, and on_neg = (alpha * -1) * po is fused in per-half STTs
            # reading the PV psum directly.
            po = [None] * HEADS
            on_sb = [sbt(f"on{t}", [P, NSEL]) for t in range(CT)]

            def emit_a2_head(jt, jw, jo, psm, upto):
                # A2 psum partial: optional Wp@bv bias + on-parts for pairs
                # < upto.  (No WpWv@xq term: the host zeroes the selected
                # query columns of x, so the main pass contributes nothing
                # at the scatter targets.)
                first = True
                if not zbias:
                    nc.tensor.matmul(psm[0:jw, :], ones1[:, 0:jw], wpbv,
                                     start=True, stop=False)
                    first = False
                for kc in range(upto):
                    nc.tensor.matmul(
                        psm[0:jw, :], on_sb[kc][:, jo:jo + jw], wp_c(kc),
                        start=first and kc == 0, stop=False,
                    )

            at_sb = [sbt(f"at{j}", [P, C]) for j in range(JT)]
            a2ps = [None] * JT
            ynext = 0
            for t in range(CT):  # head pair (2t, 2t+1)
                for half in range(2):
                    h = 2 * t + half
                    po[h] = psB.tile([HD + 1, NSEL], F32, name="pv", tag="pv")
                s_pairs = []
                for mj, (mo, mw) in enumerate(_chunks(NSEL)):
                    # the two QK matmuls run CONCURRENTLY on the PE via
                    # tile_position row-tiling (contraction is only 64 wide)
                    qkps = []
                    for half in range(2):
                        psm = psA.tile([P, NSEL], F32, name="qk", tag="qk")
                        nc.tensor.matmul(
                            psm[0:mw, :],
                            k_sb[half * HD:(half + 1) * HD,
                                 t * NSEL + mo:t * NSEL + mo + mw],
                            q_sb[half * HD:(half + 1) * HD,
                                 t * NSEL:(t + 1) * NSEL],
                            start=True, stop=True,
                            tile_position=(half * HD, 0),
                        )
                        qkps.append(psm)
                    s_pair = sbt(f"s{t}_{mj}", [P, 2 * NSEL])
                    s_pairs.append(s_pair)
                    for half in range(2):
                        nc.scalar.activation(
                            s_pair[0:mw, half * NSEL:(half + 1) * NSEL],
                            qkps[half][0:mw, :],
                            mybir.ActivationFunctionType.Exp,
                            bias=kb[0:mw, mj:mj + 1], scale=SCALE,
                        )
                    if t == 0:
                        # pair 0 has no projections/filler to absorb the
                        # exp latency; vt construction fills the PE instead
                        emit_vt(mj)
                    else:
                        if t == CT - 1 and ynext < len(ymain):
                            # the last pair has no downstream projections to
                            # absorb its exp latency; spend filler here
                            emit_ymain(ynext)
                            ynext += 1
                        for half in range(2):
                            h = 2 * t + half
                            nc.tensor.matmul(
                                po[h][:],
                                vt_sb[mj][0:mw, h * (HD + 1):(h + 1) * (HD + 1)],
                                s_pair[0:mw, half * NSEL:(half + 1) * NSEL],
                                start=(mj == 0), stop=(mj == MT - 1),
                            )
                if t == 0:
                    for mj, (mo, mw) in enumerate(_chunks(NSEL)):
                        for half in range(2):
                            h = 2 * t + half
                            nc.tensor.matmul(
                                po[h][:],
                                vt_sb[mj][0:mw, h * (HD + 1):(h + 1) * (HD + 1)],
                                s_pairs[mj][0:mw, half * NSEL:(half + 1) * NSEL],
                                start=(mj == 0), stop=(mj == MT - 1),
                            )
                # per-pair 1/Z chain (overlaps the next pair's attention).
                # The next tile's projections / filler / A2 heads sit between
                # PV and the zbc broadcast in the PE stream so the PE is
                # never waiting on the DVE-side z2 row copies.
                z2 = sbt(f"z{t}", [1, 2 * NSEL])
                for half in range(2):
                    h = 2 * t + half
                    nc.vector.tensor_copy(
                        z2[0:1, half * NSEL:(half + 1) * NSEL],
                        po[h][HD:HD + 1, :],
                    )
                if t + 1 < CT:
                    emit_proj(t + 1)
                if t in (1, 2):
                    for _ in range(3):
                        emit_ymain(ynext)
                        ynext += 1
                if t == CT - 1:
                    # A2 heads for j-tiles 0/1 (pairs 0-2 on-parts) cover the
                    # z2 -> zbc -> recip -> STT latency of the last pair
                    for jt in (0, 1):
                        jo, jw = _chunks(NSEL)[jt]
                        a2ps[jt] = cpst(C)
                        emit_a2_head(jt, jw, jo, a2ps[jt], upto=CT - 1)
                zbc = psA.tile([P, NSEL], F32, name="zbc", tag="qk")
                for half in range(2):
                    nc.tensor.matmul(
                        zbc[half * HD:(half + 1) * HD, :],
                        ones1[:, 0:HD],
                        z2[0:1, half * NSEL:(half + 1) * NSEL],
                        start=True, stop=True,
                    )
                alpbc = sbt(f"alp{t}", [P, NSEL], F32)
                nc.vector.reciprocal_approx_fast(out=alpbc[:], in_=zbc[:])
                for half in range(2):
                    h = 2 * t + half
                    nc.vector.scalar_tensor_tensor(
                        on_sb[t][half * HD:(half + 1) * HD, :],
                        alpbc[half * HD:(half + 1) * HD, :], -1.0,
                        po[h][0:HD, :],
                        op0=mybir.AluOpType.mult, op1=mybir.AluOpType.mult,
                    )

            # ---- A2^T[j, :] = (WpWv@xq + Wp@(-on) + Wp@bv)^T ----
            # j-tiles 0/1 only need the pair-3 on chunk + eviction; tile 2
            # runs in full.  Evictions ride ScalarE (idle after the last exp).
            for jt, (jo, jw) in enumerate(_chunks(NSEL)):
                if a2ps[jt] is not None:
                    psm = a2ps[jt]
                    nc.tensor.matmul(
                        psm[0:jw, :], on_sb[CT - 1][:, jo:jo + jw],
                        wp_c(CT - 1), start=False, stop=True,
                    )
                else:
                    psm = cpst(C)
                    emit_a2_head(jt, jw, jo, psm, upto=CT - 1)
                    nc.tensor.matmul(
                        psm[0:jw, :], on_sb[CT - 1][:, jo:jo + jw],
                        wp_c(CT - 1), start=False, stop=True,
                    )
                nc.scalar.activation(
                    at_sb[jt][0:jw, :], psm[0:jw, :],
                    mybir.ActivationFunctionType.Identity,
                )

            # ---- y += A2^T^T @ E_neg (column scatter of the correction) ----
            for co in range(CT):
                for nch in range(NCH):
                    psm = psB.tile([P, 512], F32, name="ye", tag="pv")
                    for jt, (jo, jw) in enumerate(_chunks(NSEL)):
                        nc.tensor.matmul(
                            psm[:],
                            at_sb[jt][0:jw, co * P:(co + 1) * P],
                            emat[0:jw, jt * N + nch * 512:jt * N + (nch + 1) * 512],
                            start=(jt == 0), stop=(jt == JT - 1),
                        )
                    seg = y_sb[:, co * N + nch * 512:co * N + (nch + 1) * 512]
                    nc.vector.tensor_tensor(
                        seg, seg, psm[:], op=mybir.AluOpType.add,
                    )
                    nc.sync.dma_start(
                        out=y_e[:]
                        .rearrange("(t p) n -> t p n", p=P)[co]
                        [:, nch * 512:(nch + 1) * 512],
                        in_=seg,
                    )

    # The greedy ACT-table-load pass alternates between exp-only and ln-only
    # table sets for our Exp/Ln/Identity/Copy mix, inserting ~9 ACT_TABLE_LOADs
    # (~1.3us each).  natural_log_exp_and_others contains all four functions;
    # make it the only candidate (list positions must stay aligned with
    # act_info.json indices, so empty the competitors instead of removing).
    import concourse.bacc as bacc_mod

    WANT = "natural_log_exp_and_others"
    orig_tables = bacc_mod.get_activation_tables

    def one_set_tables(arch):
        tabs = orig_tables(arch)
        ours = {
            mybir.ActivationFunctionType.Exp,
            mybir.ActivationFunctionType.Ln,
            mybir.ActivationFunctionType.Identity,
            mybir.ActivationFunctionType.Copy,
        }
        return {
            name: (fns if name == WANT else fns - ours)
            for name, fns in tabs.items()
        }

    bacc_mod.get_activation_tables = one_set_tables
    try:
        nc.compile()
    finally:
        bacc_mod.get_activation_tables = orig_tables
    return nc


def _get_program(NSEL, zbias):
    key = (NSEL, zbias)
    if key not in _PROGRAM_CACHE:
        _PROGRAM_CACHE[key] = _build_program(NSEL, zbias)
    return _PROGRAM_CACHE[key]


def _sel_masks(x, u, ws, bs):
    """Bit-exact replica of the reference's gumbel argmax mask (fp32, CPU jax)."""
    import jax
    import jax.numpy as jnp

    cpu = jax.devices("cpu")[0]
    with jax.default_device(cpu):
        xj = jax.device_put(jnp.asarray(x, jnp.float32), cpu)
        uj = jax.device_put(jnp.asarray(u, jnp.float32), cpu)
        wj = jax.device_put(jnp.asarray(ws, jnp.float32), cpu)
        bj = jax.device_put(jnp.asarray(bs, jnp.float32), cpu)
        logits = jnp.einsum("bchw,oc->bohw", xj, wj) + bj[None, :, None, None]
        g = -jnp.log(-jnp.log(uj + EPS) + EPS)
        m = jnp.argmax(logits + g, axis=1) == 0
        return np.asarray(m).reshape(x.shape[0], N)


def _col_layout(vec, nt):
    """[nt*128] -> [128, nt] with column t = vec[128t:128(t+1)]."""
    return np.ascontiguousarray(vec.reshape(nt, P).T)


def kernel(x, u_q, u_k, wq_s, bq_s, wk_s, bk_s, Wq, bq, Wk, bk, Wv, bv, Wp, bp):
    global LAST_RESULT
    x = np.asarray(x, np.float32)
    u_q, u_k = np.asarray(u_q, np.float32), np.asarray(u_k, np.float32)

    mq = _sel_masks(x, u_q, np.asarray(wq_s), np.asarray(bq_s))
    mk = _sel_masks(x, u_k, np.asarray(wk_s), np.asarray(bk_s))

    idx_q = [np.nonzero(mq[b])[0] for b in range(B)]
    idx_k = [np.nonzero(mk[b])[0] for b in range(B)]
    max_cnt = max(max(len(i) for i in idx_q), max(len(i) for i in idx_k))
    NSEL = NSEL_DEFAULT
    while NSEL < max_cnt:
        NSEL += 32
    MT = len(_chunks(NSEL))
    JTP = MT * P

    Wq32, Wk32 = np.asarray(Wq, np.float32), np.asarray(Wk, np.float32)
    Wv32, Wp32 = np.asarray(Wv, np.float32), np.asarray(Wp, np.float32)
    bq32, bk32 = np.asarray(bq, np.float32), np.asarray(bk, np.float32)
    bv32, bp32 = np.asarray(bv, np.float32), np.asarray(bp, np.float32)

    WpWv32 = Wp32 @ Wv32
    wpbv32 = Wp32 @ bv32
    bp2 = bp32 + wpbv32

    wqT = np.ascontiguousarray(Wq32.T)
    wkT = np.ascontiguousarray(Wk32.T)
    wvT = np.ascontiguousarray(Wv32.T).astype(BF16)
    wpT = np.ascontiguousarray(Wp32.T).astype(BF16)
    wpwvT = np.ascontiguousarray(WpWv32.T).astype(BF16)
    rows2 = np.concatenate([bv32, wpbv32]).reshape(1, 2 * C).astype(BF16)

    # consts [P, 8 + CT + MT]
    kbias_cols = MT

    xf = x.reshape(B, C, N)
    in_maps = []
    for b in range(B):
        iq, ik = idx_q[b], idx_k[b]
        cq, ck = len(iq), len(ik)
        iq_pad = np.pad(iq, (0, NSEL - cq))
        ik_pad = np.pad(ik, (0, NSEL - ck))

        kbias = np.zeros(MT * P, np.float32)
        kbias[ck:NSEL] = NEG
        kbias[NSEL:] = NEG
        consts = np.concatenate([
            _col_layout(bq32, CT), _col_layout(bk32, CT),
            _col_layout(bp2, CT),
            _col_layout(kbias, MT)[:, :kbias_cols],
        ], axis=1).astype(np.float32)
        consts = np.ascontiguousarray(consts)

        emat = np.zeros((JTP, N), BF16)
        emat[np.arange(cq), iq[:cq]] = -1.0

        xb = xf[b].copy()
        xb[:, iq] = 0.0            # main pass must not contribute at scatter targets
        xb = xb.astype(BF16)
        xqg = np.ascontiguousarray(xf[b][:, iq_pad]).astype(BF16)
        xkg = np.ascontiguousarray(xf[b][:, ik_pad]).astype(BF16)
        wqxq = np.ascontiguousarray(
            np.concatenate([xf[b][:, iq_pad], wqT], axis=1)).astype(BF16)
        wkxk = np.ascontiguousarray(
            np.concatenate([xf[b][:, ik_pad], wkT], axis=1)).astype(BF16)
        del xqg, xkg

        in_maps.append({
            "consts": consts,
            "rows2": rows2,
            "wqxq": wqxq,
            "wkxk": wkxk,
            "wvT": wvT,
            "xbf": xb,
            "wpwvT": wpwvT,
            "wpT": wpT,
            "emat": emat,
        })

    zbias = not (bq32.any() or bk32.any() or bv32.any() or bp32.any())
    nc = _get_program(NSEL, zbias)
    res = run_bass_kernel_spmd(nc, in_maps, list(range(B)), trace=TRACE)
    LAST_RESULT = res

    y = np.stack([np.asarray(res.results[b]["y"], np.float32) for b in range(B)])
    return y.reshape(B, C, H, W)


# revision 31
# speedup vs baseline: 1.0397x; 1.0177x over previous
"""Trainium2 Bass kernel for gumbel-masked sparse attention.

Problem (hardcoded shapes): B=8, C=512, H=W=32 (N=1024), heads=8, hd=64, R=4.

    mq/mk  = (argmax over R of conv1x1(x, w*_s) + gumbel(u), axis=1) == 0
    q/k/v  = conv1x1(x, W*, b*)
    attn   = softmax over selected keys of (q^T k) * hd^-0.5
    out    = where(mq, attn @ v, v);  y = conv1x1(out, Wp, bp)

Distribution: data-parallel over batch B across the 8 NeuronCores (one
batch element per core), weights replicated.  The gumbel argmax masks are
computed on host (they must match the reference's fp32 CPU semantics
bit-for-bit — a single flipped mask position discretely changes a whole
output column), and the device kernel exploits the ~1/4 sparsity:
attention runs only on the selected (gathered, padded-to-NSEL) query/key
positions.

Formulation (avoids the baseline's full-N Wv@x pass):
    y = (Wp Wv) @ x + (Wp bv + bp)  +  A2^T @ E_neg
    A2^T[j, :] = (WpWv @ xq + Wp @ (-on) + Wp bv)[:, j]^T
    E_neg[j, iq[j]] = -1   (for j < count(mq))
so the correction  Wp@on - WpWv@xq - Wp@bv  lands exactly on the selected
query columns.  WpWv, Wp@bv and bp2 = bp + Wp@bv are precomputed on host.

Perf notes (from ntff traces of the previous version): each dma_start
costs ~600ns of *issue* time on the Sync engine regardless of size, so
all inputs are coalesced into 9 large first-use-ordered DMAs; the
per-pair softmax 1/Z partition broadcast is a 2-contraction matmul
instead of a DRAM bounce (removes 12 DMAs from the critical path); the
WpWv@x main-pass groups are emitted between attention pairs as PE filler.
"""

import numpy as np
import ml_dtypes

import concourse.bacc as bacc
import concourse.mybir as mybir
import concourse.tile as tile
from concourse.bass_utils import run_bass_kernel_spmd

BF16 = ml_dtypes.bfloat16
F32 = mybir.dt.float32
BF = mybir.dt.bfloat16

B, C, H, W = 8, 512, 32, 32
N = H * W                      # 1024
HEADS, HD = 8, 64
SCALE = HD ** -0.5             # 0.125
EPS = 1e-10
NEG = -30000.0                 # additive key-mask bias; exp(NEG + x) == 0
P = 128
CT = C // P                    # 4 channel tiles
NCH = N // 512                 # 2 free-dim chunks of the full N

NSEL_DEFAULT = 288             # padded selected-position count (max count 277)

TRACE = False                  # set True from test harness to profile
LAST_RESULT = None             # BassKernelResults of the last run (for tests)

_PROGRAM_CACHE = {}


# Drop the second all-engine barrier of TileContext's exit sequence
# (drain -> barrier -> sem clears -> barrier).  The gpsimd sem-clear stream
# still completes before the NEFF finishes (every engine stream must end),
# and no instruction follows it, so the final cross-engine alignment only
# adds ~3-4us of EVSEM butterfly to every execution.
def _slim_drain_and_barrier(self, tick_clock, wait_clock):
    from concourse.vector_clock import ScopedClock

    drain_inst = self.nc.sync.drain()
    wait_clock.add_sem_waits(
        drain_inst.ins, ScopedClock({None: tick_clock.global_clock})
    )
    self.nc.all_engine_barrier()
    popped = self.nc._tile_sem_poison_stack.pop()
    assert popped is self._sem_poison
    self.nc.clear_and_free_semaphores(list(self.sems.allocated().values()))


tile.TileContext._drain_and_barrier = _slim_drain_and_barrier



def _chunks(total, step=P):
    return [(o, min(step, total - o)) for o in range(0, total, step)]


def _build_program(NSEL, zbias):
    MT = len(_chunks(NSEL))    # m-chunks over selected keys
    JT = MT                    # j-tiles over selected queries
    JTP = JT * P               # emat row padding
    WQX = C + NSEL             # width of the wq|xq (wk|xk) bundles
    NC_ = 8 + CT + 2 * MT      # consts width
    nc = bacc.Bacc("TRN2", target_bir_lowering=False, debug=False, num_devices=8)

    # consts layout [P, NC_] f32:
    #   cols 0:4 bq, 4:8 bk, 8:12 bp2 (=bp + Wp@bv), 12:12+MT kbias
    consts_e = nc.declare_dram_parameter("consts", [P, NC_], F32, isOutput=False)
    rows2_e = nc.declare_dram_parameter("rows2", [1, 2 * C], BF, isOutput=False)
    wqxq_e = nc.declare_dram_parameter("wqxq", [C, WQX], BF, isOutput=False)
    wkxk_e = nc.declare_dram_parameter("wkxk", [C, WQX], BF, isOutput=False)
    wv_e = nc.declare_dram_parameter("wvT", [C, C], BF, isOutput=False)
    x_e = nc.declare_dram_parameter("xbf", [C, N], BF, isOutput=False)
    wpwv_e = nc.declare_dram_parameter("wpwvT", [C, C], BF, isOutput=False)
    wp_e = nc.declare_dram_parameter("wpT", [C, C], BF, isOutput=False)
    y_e = nc.declare_dram_parameter("y", [C, N], BF, isOutput=True)

    def folded(ap):
        # [t*128, w] dram -> [128, t, w] (lands in sbuf as chunk kc at cols kc*w)
        return ap[:].rearrange("(t p) n -> p t n", p=P)

    def unfold(sb_tile, t):
        # [128, t*w] sbuf tile viewed [128, t, w] to match folded(dram)
        return sb_tile[:].rearrange("p (t n) -> p t n", t=t)

    with tile.TileContext(nc) as tc:
        with (
            tc.tile_pool(name="sb", bufs=1) as sb,
            tc.tile_pool(name="psA", bufs=4, space="PSUM") as psA,
            tc.tile_pool(name="psB", bufs=2, space="PSUM") as psB,
            tc.tile_pool(name="psC", bufs=2, space="PSUM") as psC,
        ):
            def sbt(tag, shape, dtype=BF):
                return sb.tile(shape, dtype, name=tag, tag=tag)

            def cpst(w=512):
                return psC.tile([P, w], F32, name="mm", tag="mm")

            # ---- coalesced input DMAs, first-use order; the two big
            # critical bundles go first on the sync HWDGE queue, the small
            # consts ride the scalar HWDGE queue in parallel ----
            # q/k bundles are split: the gathered data + tile-0 weight
            # columns stream first so pair 0's projections start ~4us
            # earlier; the remaining weight tiles follow wv.
            HEADW = NSEL + P
            wqxq = sbt("wqxq", [P, CT * WQX])
            wkxk = sbt("wkxk", [P, CT * WQX])
            nc.sync.dma_start(out=unfold(wqxq, CT)[:, :, 0:HEADW],
                              in_=folded(wqxq_e)[:, :, 0:HEADW])
            nc.sync.dma_start(out=unfold(wkxk, CT)[:, :, 0:HEADW],
                              in_=folded(wkxk_e)[:, :, 0:HEADW])
            consts = sbt("consts", [P, NC_], F32)
            nc.scalar.dma_start(out=consts[:], in_=consts_e[:])
            rows2 = sbt("rows2", [1, 2 * C])
            nc.scalar.dma_start(out=rows2[:], in_=rows2_e[:])
            wv = sbt("wv", [P, CT * C])
            nc.sync.dma_start(out=unfold(wv, CT), in_=folded(wv_e))
            nc.sync.dma_start(out=unfold(wqxq, CT)[:, :, HEADW:WQX],
                              in_=folded(wqxq_e)[:, :, HEADW:WQX])
            nc.sync.dma_start(out=unfold(wkxk, CT)[:, :, HEADW:WQX],
                              in_=folded(wkxk_e)[:, :, HEADW:WQX])
            x = sbt("x", [P, CT * N])
            nc.sync.dma_start(out=unfold(x, CT), in_=folded(x_e))
            wpwv = sbt("wpwv", [P, CT * C])
            nc.sync.dma_start(out=unfold(wpwv, CT), in_=folded(wpwv_e))
            wp = sbt("wp", [P, CT * C])
            nc.sync.dma_start(out=unfold(wp, CT), in_=folded(wp_e))

            bq = consts[:, 0:CT]
            bk = consts[:, CT:2 * CT]
            bp2 = consts[:, 2 * CT:3 * CT]
            kb = consts[:, 3 * CT:3 * CT + MT]
            iqc = consts[:, 3 * CT + MT:3 * CT + 2 * MT]
            bvrow = rows2[0:1, 0:C]
            wpbv = rows2[0:1, C:2 * C]

            def xq_c(kc):
                return wqxq[:, kc * WQX:kc * WQX + NSEL]

            def wq_c(kc):
                return wqxq[:, kc * WQX + NSEL:(kc + 1) * WQX]

            def xk_c(kc):
                return wkxk[:, kc * WQX:kc * WQX + NSEL]

            def wk_c(kc):
                return wkxk[:, kc * WQX + NSEL:(kc + 1) * WQX]

            def wv_c(kc):
                return wv[:, kc * C:(kc + 1) * C]

            def wpwv_c(kc):
                return wpwv[:, kc * C:(kc + 1) * C]

            def wp_c(kc):
                return wp[:, kc * C:(kc + 1) * C]

            ones1 = sbt("ones1", [1, P])
            nc.vector.memset(ones1[:], 1.0)

            # dummy activation with no data deps: pulls the ACT_TABLE_LOAD
            # (~1.3us) to t=0 instead of serializing before the first real exp
            warm = sbt("warm", [1, 1], F32)
            nc.vector.memset(warm[:], 1.0)
            nc.scalar.activation(warm[:], warm[:], mybir.ActivationFunctionType.Exp)

            # dummy matmuls while the input DMAs land: sustained PE activity
            # flips the HAM clock gate to 2.4 GHz before real work
            wmm = sbt("wmm", [P, 512])
            nc.vector.memset(wmm[:], 0.0)
            for _ in range(5):
                wps = psC.tile([P, 512], F32, name="wps", tag="mm")
                nc.tensor.matmul(wps[:], wmm[:, :P], wmm[:], start=True, stop=True)

            # ---- scatter matrix E[j, n] = (iq[j] == n), built from a
            # GPSIMD iota row + per-partition DVE compares in DVE's idle
            # early window (saves a 768KB DMA); padded j rows use iq = -1
            # so they never match.
            # The sign of the correction is folded into the at eviction
            # (scale=-1), so E holds +1.
            emat = sbt("emat", [P, JT * N])
            it128 = sbt("iota", [P, N], F32)
            nc.gpsimd.iota(it128[:], pattern=[[1, N]], channel_multiplier=0,
                           allow_small_or_imprecise_dtypes=True)
            for jt in range(JT):
                nc.vector.tensor_scalar(
                    emat[:, jt * N:(jt + 1) * N], it128[:],
                    iqc[:, jt:jt + 1], None, op0=mybir.AluOpType.is_equal,
                )

            # ---- q/k projections: [C, NSEL] bf16; tile t+1 is emitted
            # after pair t's attention (weights for tiles 1-3 stream late)
            q_sb = sbt("q", [P, CT * NSEL])
            k_sb = sbt("k", [P, CT * NSEL])

            def emit_proj(t):
                # q and k chains interleave so each adjacent PE instruction
                # targets a different psum bank (accumulating back-to-back
                # into one bank serializes at ~2x the streaming cost)
                psq, psk = cpst(NSEL), cpst(NSEL)
                for kc in range(CT):
                    nc.tensor.matmul(
                        psq[:], wq_c(kc)[:, t * P:(t + 1) * P], xq_c(kc),
                        start=(kc == 0), stop=(kc == CT - 1),
                    )
                    nc.tensor.matmul(
                        psk[:], wk_c(kc)[:, t * P:(t + 1) * P], xk_c(kc),
                        start=(kc == 0), stop=(kc == CT - 1),
                    )
                nc.vector.tensor_scalar_add(
                    q_sb[:, t * NSEL:(t + 1) * NSEL], psq[:], bq[:, t:t + 1],
                )
                nc.vector.tensor_scalar_add(
                    k_sb[:, t * NSEL:(t + 1) * NSEL], psk[:], bk[:, t:t + 1],
                )

            emit_proj(0)

            # ---- vT_sel[m, 65h + d] = v_sel[64h + d, m]; column 65h+64 = 1.0
            # (ones column makes the PV matmul also produce Z = sum_m S[m, j])
            # Emitted inside pair 0's QK stream: pair 0 has no other PE
            # filler while ACT works through its exps.
            vt_sb = [sbt(f"vt{mt}", [P, HEADS * (HD + 1)]) for mt in range(MT)]

            def emit_vt(*mts):
                ps = {}
                for mt in mts:
                    ps[mt] = cpst(C)
                for kc in range(CT):
                    for mt in mts:
                        mo, mw = _chunks(NSEL)[mt]
                        nc.tensor.matmul(
                            ps[mt][0:mw, :],
                            xk_c(kc)[:, mo:mo + mw],
                            wv_c(kc),
                            start=(kc == 0), stop=(zbias and kc == CT - 1),
                        )
                for mt in mts:
                    mo, mw = _chunks(NSEL)[mt]
                    psm = ps[mt]
                    if not zbias:
                        nc.tensor.matmul(psm[0:mw, :], ones1[:, 0:mw], bvrow,
                                         start=False, stop=True)
                    vt_view = vt_sb[mt][:].rearrange("p (h d) -> p h d", d=HD + 1)
                    nc.vector.tensor_copy(
                        vt_view[0:mw, :, 0:HD],
                        psm[0:mw, :].rearrange("p (h d) -> p h d", d=HD),
                    )
                    nc.vector.memset(vt_view[0:mw, :, HD:HD + 1], 1.0)

            # ---- WpWv@x main-pass groups (PE filler between pairs) ----
            y_sb = sbt("y", [P, CT * N])
            ymain = [(co, nch) for co in range(CT) for nch in range(NCH)]

            def emit_ymain2(g0, g1):
                # two groups interleaved across two psum banks
                gs = [g for g in (g0, g1) if g is not None and g < len(ymain)]
                ps = [psA.tile([P, 512], F32, name="ym", tag="qk") for _ in gs]
                for kc in range(CT):
                    for gi, psm in zip(gs, ps):
                        co, nch = ymain[gi]
                        nc.tensor.matmul(
                            psm[:],
                            wpwv_c(kc)[:, co * P:(co + 1) * P],
                            x[:, kc * N + nch * 512:kc * N + (nch + 1) * 512],
                            start=(kc == 0), stop=(kc == CT - 1),
                        )
                for gi, psm in zip(gs, ps):
                    co, nch = ymain[gi]
                    seg = y_sb[:, co * N + nch * 512:co * N + (nch + 1) * 512]
                    nc.vector.tensor_scalar_add(seg, psm[:], bp2[:, co:co + 1])

            # ---- attention (selected keys m in partitions, queries j free) ----
            # S[m, j] = exp(scale * k_m . q_j + kbias[m]), bf16.  Per pair:
            # Z sits in po row 64 (vt ones column); two DVE row-copies pull it
            # to SBUF, a ones-matmul broadcasts it across the head partitions,
            # reciprocal_approx_fast (DVE, ~18 bits) gives alpha with no ACT
            # work, and on_neg = (alpha * -1) * po is fused in per-half STTs
            # reading the PV psum directly.
            po = [None] * HEADS
            on_sb = [sbt(f"on{t}", [P, NSEL]) for t in range(CT)]

            def emit_a2_head(jt, jw, jo, psm, upto):
                # A2 psum partial: optional Wp@bv bias + on-parts for pairs
                # < upto.  (No WpWv@xq term: the host zeroes the selected
                # query columns of x, so the main pass contributes nothing
                # at the scatter targets.)
                first = True
                if not zbias:
                    nc.tensor.matmul(psm[0:jw, :], ones1[:, 0:jw], wpbv,
                                     start=True, stop=False)
                    first = False
                for kc in range(upto):
                    nc.tensor.matmul(
                        psm[0:jw, :], on_sb[kc][:, jo:jo + jw], wp_c(kc),
                        start=first and kc == 0, stop=False,
                    )

            at_sb = [sbt(f"at{j}", [P, C]) for j in range(JT)]
            a2ps = [None] * JT
            ynext = 0
            for t in range(CT):  # head pair (2t, 2t+1)
                for half in range(2):
                    h = 2 * t + half
                    po[h] = psB.tile([HD + 1, NSEL], F32, name="pv", tag="pv")
                s_pairs = []
                for mj, (mo, mw) in enumerate(_chunks(NSEL)):
                    # the two QK matmuls run CONCURRENTLY on the PE via
                    # tile_position row-tiling (contraction is only 64 wide)
                    qkps = []
                    for half in range(2):
                        psm = psA.tile([P, NSEL], F32, name="qk", tag="qk")
                        nc.tensor.matmul(
                            psm[0:mw, :],
                            k_sb[half * HD:(half + 1) * HD,
                                 t * NSEL + mo:t * NSEL + mo + mw],
                            q_sb[half * HD:(half + 1) * HD,
                                 t * NSEL:(t + 1) * NSEL],
                            start=True, stop=True,
                            tile_position=(half * HD, 0),
                        )
                        qkps.append(psm)
                    s_pair = sbt(f"s{t}_{mj}", [P, 2 * NSEL])
                    s_pairs.append(s_pair)
                    for half in range(2):
                        nc.scalar.activation(
                            s_pair[0:mw, half * NSEL:(half + 1) * NSEL],
                            qkps[half][0:mw, :],
                            mybir.ActivationFunctionType.Exp,
                            bias=kb[0:mw, mj:mj + 1],
                        )
                    if t == 0:
                        # pair 0 has no projections/filler to absorb the
                        # exp latency; vt construction fills the PE instead
                        if mj == 1:
                            emit_vt(0, 1)
                        elif mj == 2:
                            emit_vt(2)
                    else:
                        if t == CT - 1 and mj == 0 and ynext < len(ymain):
                            # the last pair has no downstream projections to
                            # absorb its exp latency; spend filler here
                            emit_ymain2(ynext, ynext + 1)
                            ynext += 2
                        for half in range(2):
                            h = 2 * t + half
                            nc.tensor.matmul(
                                po[h][:],
                                vt_sb[mj][0:mw, h * (HD + 1):(h + 1) * (HD + 1)],
                                s_pair[0:mw, half * NSEL:(half + 1) * NSEL],
                                start=(mj == 0), stop=(mj == MT - 1),
                            )
                if t == 0:
                    for mj, (mo, mw) in enumerate(_chunks(NSEL)):
                        for half in range(2):
                            h = 2 * t + half
                            nc.tensor.matmul(
                                po[h][:],
                                vt_sb[mj][0:mw, h * (HD + 1):(h + 1) * (HD + 1)],
                                s_pairs[mj][0:mw, half * NSEL:(half + 1) * NSEL],
                                start=(mj == 0), stop=(mj == MT - 1),
                            )
                # per-pair 1/Z chain (overlaps the next pair's attention).
                # The next tile's projections / filler / A2 heads sit between
                # PV and the zbc broadcast in the PE stream so the PE is
                # never waiting on the DVE-side z2 row copies.
                z2 = sbt(f"z{t}", [1, 2 * NSEL])
                for half in range(2):
                    h = 2 * t + half
                    zseg = z2[0:1, half * NSEL:(half + 1) * NSEL]
                    if t == CT - 1 and half == 0:
                        # tail-critical: run the two row copies concurrently
                        # on ACT and DVE (ACT's exps are all done by now)
                        nc.scalar.activation(
                            zseg, po[h][HD:HD + 1, :],
                            mybir.ActivationFunctionType.Identity,
                        )
                    else:
                        nc.vector.tensor_copy(zseg, po[h][HD:HD + 1, :])
                if t + 1 < CT:
                    emit_proj(t + 1)
                if t in (1, 2):
                    emit_ymain2(ynext, ynext + 1)
                    ynext += 2
                if t == CT - 1:
                    # A2 heads for j-tiles 0/1 (pairs 0-2 on-parts) cover the
                    # z2 -> zbc -> recip -> STT latency of the last pair;
                    # the two chains interleave across psum banks
                    for jt in (0, 1):
                        a2ps[jt] = cpst(C)
                    first = [True, True]
                    if not zbias:
                        for jt in (0, 1):
                            jo, jw = _chunks(NSEL)[jt]
                            nc.tensor.matmul(
                                a2ps[jt][0:jw, :], ones1[:, 0:jw], wpbv,
                                start=True, stop=False)
                            first[jt] = False
                    for kc in range(CT - 1):
                        for jt in (0, 1):
                            jo, jw = _chunks(NSEL)[jt]
                            nc.tensor.matmul(
                                a2ps[jt][0:jw, :],
                                on_sb[kc][:, jo:jo + jw], wp_c(kc),
                                start=first[jt] and kc == 0, stop=False,
                            )
                zbc = psA.tile([P, NSEL], F32, name="zbc", tag="qk")
                for half in range(2):
                    nc.tensor.matmul(
                        zbc[half * HD:(half + 1) * HD, :],
                        ones1[:, 0:HD],
                        z2[0:1, half * NSEL:(half + 1) * NSEL],
                        start=True, stop=True,
                    )
                if t == CT - 1 and ynext < len(ymain):
                    emit_ymain2(ynext, ynext + 1)  # covers recip+STT latency
                    ynext += 2
                alpbc = sbt(f"alp{t}", [P, NSEL], F32)
                nc.vector.reciprocal_approx_fast(out=alpbc[:], in_=zbc[:])
                for half in range(2):
                    h = 2 * t + half
                    nc.vector.scalar_tensor_tensor(
                        on_sb[t][half * HD:(half + 1) * HD, :],
                        alpbc[half * HD:(half + 1) * HD, :], -1.0,
                        po[h][0:HD, :],
                        op0=mybir.AluOpType.mult, op1=mybir.AluOpType.mult,
                    )

            # ---- A2^T[j, :] = (Wp@(-on) + Wp@bv)^T, negated at eviction --
            # j-tiles 0/1 only need the pair-3 on chunk; tile 2 runs in full
            # on a psB bank, its chain interleaved with the 0/1 finishers.
            # Evictions ride ScalarE (idle after the last exp).
            jo2, jw2 = _chunks(NSEL)[2]
            a2ps[2] = psB.tile([P, C], F32, name="pv", tag="pv")
            first2 = True
            if not zbias:
                nc.tensor.matmul(a2ps[2][0:jw2, :], ones1[:, 0:jw2], wpbv,
                                 start=True, stop=False)
                first2 = False
            for kc in range(CT):
                nc.tensor.matmul(
                    a2ps[2][0:jw2, :], on_sb[kc][:, jo2:jo2 + jw2], wp_c(kc),
                    start=first2 and kc == 0, stop=(kc == CT - 1),
                )
                if kc < 2:
                    jo, jw = _chunks(NSEL)[kc]
                    nc.tensor.matmul(
                        a2ps[kc][0:jw, :], on_sb[CT - 1][:, jo:jo + jw],
                        wp_c(CT - 1), start=False, stop=True,
                    )
            for jt, (jo, jw) in enumerate(_chunks(NSEL)):
                if jt == 0:
                    # DVE and ACT evict concurrently; YE only gates on at0
                    nc.vector.tensor_scalar_mul(
                        at_sb[jt][0:jw, :], a2ps[jt][0:jw, :], -1.0)
                else:
                    nc.scalar.activation(
                        at_sb[jt][0:jw, :], a2ps[jt][0:jw, :],
                        mybir.ActivationFunctionType.Identity, scale=-1.0,
                    )

            # ---- y += at^T @ E (column scatter of the correction) ----
            # groups run in interleaved pairs across two psum banks
            for co in range(CT):
                ps = [psB.tile([P, 512], F32, name="ye", tag="pv")
                      for _ in range(NCH)]
                for jt, (jo, jw) in enumerate(_chunks(NSEL)):
                    for nch in range(NCH):
                        nc.tensor.matmul(
                            ps[nch][:],
                            at_sb[jt][0:jw, co * P:(co + 1) * P],
                            emat[0:jw, jt * N + nch * 512:jt * N + (nch + 1) * 512],
                            start=(jt == 0), stop=(jt == JT - 1),
                        )
                for nch in range(NCH):
                    seg = y_sb[:, co * N + nch * 512:co * N + (nch + 1) * 512]
                    nc.vector.tensor_tensor(
                        seg, seg, ps[nch][:], op=mybir.AluOpType.add,
                    )
                    nc.scalar.dma_start(
                        out=y_e[:]
                        .rearrange("(t p) n -> t p n", p=P)[co]
                        [:, nch * 512:(nch + 1) * 512],
                        in_=seg,
                    )

    # The greedy ACT-table-load pass alternates between exp-only and ln-only
    # table sets for our Exp/Ln/Identity/Copy mix, inserting ~9 ACT_TABLE_LOADs
    # (~1.3us each).  natural_log_exp_and_others contains all four functions;
    # make it the only candidate (list positions must stay aligned with
    # act_info.json indices, so empty the competitors instead of removing).
    import concourse.bacc as bacc_mod

    WANT = "natural_log_exp_and_others"
    orig_tables = bacc_mod.get_activation_tables

    def one_set_tables(arch):
        tabs = orig_tables(arch)
        ours = {
            mybir.ActivationFunctionType.Exp,
            mybir.ActivationFunctionType.Ln,
            mybir.ActivationFunctionType.Identity,
            mybir.ActivationFunctionType.Copy,
        }
        return {
            name: (fns if name == WANT else fns - ours)
            for name, fns in tabs.items()
        }

    bacc_mod.get_activation_tables = one_set_tables
    try:
        nc.compile()
    finally:
        bacc_mod.get_activation_tables = orig_tables
    return nc


def _get_program(NSEL, zbias):
    key = (NSEL, zbias)
    if key not in _PROGRAM_CACHE:
        _PROGRAM_CACHE[key] = _build_program(NSEL, zbias)
    return _PROGRAM_CACHE[key]


def _sel_masks(x, u, ws, bs):
    """Bit-exact replica of the reference's gumbel argmax mask (fp32, CPU jax)."""
    import jax
    import jax.numpy as jnp

    cpu = jax.devices("cpu")[0]
    with jax.default_device(cpu):
        xj = jax.device_put(jnp.asarray(x, jnp.float32), cpu)
        uj = jax.device_put(jnp.asarray(u, jnp.float32), cpu)
        wj = jax.device_put(jnp.asarray(ws, jnp.float32), cpu)
        bj = jax.device_put(jnp.asarray(bs, jnp.float32), cpu)
        logits = jnp.einsum("bchw,oc->bohw", xj, wj) + bj[None, :, None, None]
        g = -jnp.log(-jnp.log(uj + EPS) + EPS)
        m = jnp.argmax(logits + g, axis=1) == 0
        return np.asarray(m).reshape(x.shape[0], N)


def _col_layout(vec, nt):
    """[nt*128] -> [128, nt] with column t = vec[128t:128(t+1)]."""
    return np.ascontiguousarray(vec.reshape(nt, P).T)


def kernel(x, u_q, u_k, wq_s, bq_s, wk_s, bk_s, Wq, bq, Wk, bk, Wv, bv, Wp, bp):
    global LAST_RESULT
    x = np.asarray(x, np.float32)
    u_q, u_k = np.asarray(u_q, np.float32), np.asarray(u_k, np.float32)

    mq = _sel_masks(x, u_q, np.asarray(wq_s), np.asarray(bq_s))
    mk = _sel_masks(x, u_k, np.asarray(wk_s), np.asarray(bk_s))

    idx_q = [np.nonzero(mq[b])[0] for b in range(B)]
    idx_k = [np.nonzero(mk[b])[0] for b in range(B)]
    max_cnt = max(max(len(i) for i in idx_q), max(len(i) for i in idx_k))
    NSEL = NSEL_DEFAULT
    while NSEL < max_cnt:
        NSEL += 32
    MT = len(_chunks(NSEL))
    JTP = MT * P

    Wq32, Wk32 = np.asarray(Wq, np.float32), np.asarray(Wk, np.float32)
    Wv32, Wp32 = np.asarray(Wv, np.float32), np.asarray(Wp, np.float32)
    bq32 = np.asarray(bq, np.float32)
    bk32 = np.asarray(bk, np.float32) * SCALE
    bv32, bp32 = np.asarray(bv, np.float32), np.asarray(bp, np.float32)

    WpWv32 = Wp32 @ Wv32
    wpbv32 = Wp32 @ bv32
    bp2 = bp32 + wpbv32

    wqT = np.ascontiguousarray(Wq32.T)
    wkT = np.ascontiguousarray(Wk32.T) * SCALE   # fold the softmax scale into k
    wvT = np.ascontiguousarray(Wv32.T).astype(BF16)
    wpT = np.ascontiguousarray(Wp32.T).astype(BF16)
    wpwvT = np.ascontiguousarray(WpWv32.T).astype(BF16)
    rows2 = np.concatenate([bv32, wpbv32]).reshape(1, 2 * C).astype(BF16)

    # consts [P, 8 + CT + MT]
    kbias_cols = MT

    xf = x.reshape(B, C, N)
    in_maps = []
    for b in range(B):
        iq, ik = idx_q[b], idx_k[b]
        cq, ck = len(iq), len(ik)
        iq_pad = np.pad(iq, (0, NSEL - cq))
        ik_pad = np.pad(ik, (0, NSEL - ck))

        kbias = np.zeros(MT * P, np.float32)
        kbias[ck:NSEL] = NEG
        kbias[NSEL:] = NEG
        iqf = np.full(MT * P, -1.0, np.float32)
        iqf[:cq] = iq.astype(np.float32)
        consts = np.concatenate([
            _col_layout(bq32, CT), _col_layout(bk32, CT),
            _col_layout(bp2, CT),
            _col_layout(kbias, MT)[:, :kbias_cols],
            _col_layout(iqf, MT),
        ], axis=1).astype(np.float32)
        consts = np.ascontiguousarray(consts)

        xb = xf[b].copy()
        xb[:, iq] = 0.0            # main pass must not contribute at scatter targets
        xb = xb.astype(BF16)
        xqg = np.ascontiguousarray(xf[b][:, iq_pad]).astype(BF16)
        xkg = np.ascontiguousarray(xf[b][:, ik_pad]).astype(BF16)
        wqxq = np.ascontiguousarray(
            np.concatenate([xf[b][:, iq_pad], wqT], axis=1)).astype(BF16)
        wkxk = np.ascontiguousarray(
            np.concatenate([xf[b][:, ik_pad], wkT], axis=1)).astype(BF16)
        del xqg, xkg

        in_maps.append({
            "consts": consts,
            "rows2": rows2,
            "wqxq": wqxq,
            "wkxk": wkxk,
            "wvT": wvT,
            "xbf": xb,
            "wpwvT": wpwvT,
            "wpT": wpT,
        })

    zbias = not (bq32.any() or bk32.any() or bv32.any() or bp32.any())
    nc = _get_program(NSEL, zbias)
    res = run_bass_kernel_spmd(nc, in_maps, list(range(B)), trace=TRACE)
    LAST_RESULT = res

    y = np.stack([np.asarray(res.results[b]["y"], np.float32) for b in range(B)])
    return y.reshape(B, C, H, W)


# revision 34
# speedup vs baseline: 1.0911x; 1.0494x over previous
"""Trainium2 Bass kernel for gumbel-masked sparse attention.

Problem (hardcoded shapes): B=8, C=512, H=W=32 (N=1024), heads=8, hd=64, R=4.

    mq/mk  = (argmax over R of conv1x1(x, w*_s) + gumbel(u), axis=1) == 0
    q/k/v  = conv1x1(x, W*, b*)
    attn   = softmax over selected keys of (q^T k) * hd^-0.5
    out    = where(mq, attn @ v, v);  y = conv1x1(out, Wp, bp)

Distribution: data-parallel over batch B across the 8 NeuronCores (one
batch element per core), weights replicated.  The gumbel argmax masks are
computed on host (they must match the reference's fp32 CPU semantics
bit-for-bit — a single flipped mask position discretely changes a whole
output column), and the device kernel exploits the ~1/4 sparsity:
attention runs only on the selected (gathered, padded-to-NSEL) query/key
positions.

Formulation (avoids the baseline's full-N Wv@x pass):
    y = (Wp Wv) @ x + (Wp bv + bp)  +  A2^T @ E_neg
    A2^T[j, :] = (WpWv @ xq + Wp @ (-on) + Wp bv)[:, j]^T
    E_neg[j, iq[j]] = -1   (for j < count(mq))
so the correction  Wp@on - WpWv@xq - Wp@bv  lands exactly on the selected
query columns.  WpWv, Wp@bv and bp2 = bp + Wp@bv are precomputed on host.

Perf notes (from ntff traces of the previous version): each dma_start
costs ~600ns of *issue* time on the Sync engine regardless of size, so
all inputs are coalesced into 9 large first-use-ordered DMAs; the
per-pair softmax 1/Z partition broadcast is a 2-contraction matmul
instead of a DRAM bounce (removes 12 DMAs from the critical path); the
WpWv@x main-pass groups are emitted between attention pairs as PE filler.
"""

import numpy as np
import ml_dtypes

import concourse.bacc as bacc
import concourse.mybir as mybir
import concourse.tile as tile
from concourse.bass_utils import run_bass_kernel_spmd

BF16 = ml_dtypes.bfloat16
F32 = mybir.dt.float32
BF = mybir.dt.bfloat16

B, C, H, W = 8, 512, 32, 32
N = H * W                      # 1024
HEADS, HD = 8, 64
SCALE = HD ** -0.5             # 0.125
EPS = 1e-10
NEG = -30000.0                 # additive key-mask bias; exp(NEG + x) == 0
P = 128
CT = C // P                    # 4 channel tiles
NCH = N // 512                 # 2 free-dim chunks of the full N

NSEL_DEFAULT = 288             # padded selected-position count (max count 277)

TRACE = False                  # set True from test harness to profile
LAST_RESULT = None             # BassKernelResults of the last run (for tests)

_PROGRAM_CACHE = {}


# Drop the second all-engine barrier of TileContext's exit sequence
# (drain -> barrier -> sem clears -> barrier).  The gpsimd sem-clear stream
# still completes before the NEFF finishes (every engine stream must end),
# and no instruction follows it, so the final cross-engine alignment only
# adds ~3-4us of EVSEM butterfly to every execution.
def _slim_drain_and_barrier(self, tick_clock, wait_clock):
    from concourse.vector_clock import ScopedClock

    drain_inst = self.nc.sync.drain()
    wait_clock.add_sem_waits(
        drain_inst.ins, ScopedClock({None: tick_clock.global_clock})
    )
    self.nc.all_engine_barrier()
    popped = self.nc._tile_sem_poison_stack.pop()
    assert popped is self._sem_poison
    self.nc.clear_and_free_semaphores(list(self.sems.allocated().values()))


tile.TileContext._drain_and_barrier = _slim_drain_and_barrier



def _chunks(total, step=P):
    return [(o, min(step, total - o)) for o in range(0, total, step)]


def _build_program(NSEL, zbias):
    MT = len(_chunks(NSEL))    # m-chunks over selected keys
    JT = MT                    # j-tiles over selected queries
    JTP = JT * P               # emat row padding
    WQX = C + NSEL             # width of the wq|xq (wk|xk) bundles
    NC_ = 8 + CT + 2 * MT      # consts width
    nc = bacc.Bacc("TRN2", target_bir_lowering=False, debug=False, num_devices=8)

    # consts layout [P, NC_] f32:
    #   cols 0:4 bq, 4:8 bk, 8:12 bp2 (=bp + Wp@bv), 12:12+MT kbias
    consts_e = nc.declare_dram_parameter("consts", [P, NC_], F32, isOutput=False)
    rows2_e = nc.declare_dram_parameter("rows2", [1, 2 * C], BF, isOutput=False)
    wqxq_e = nc.declare_dram_parameter("wqxq", [C, WQX], BF, isOutput=False)
    wkxk_e = nc.declare_dram_parameter("wkxk", [C, WQX], BF, isOutput=False)
    wv_e = nc.declare_dram_parameter("wvT", [C, C], BF, isOutput=False)
    x_e = nc.declare_dram_parameter("xbf", [C, N], BF, isOutput=False)
    wpwv_e = nc.declare_dram_parameter("wpwvT", [C, C], BF, isOutput=False)
    wp_e = nc.declare_dram_parameter("wpT", [C, C], BF, isOutput=False)
    y_e = nc.declare_dram_parameter("y", [C, N], BF, isOutput=True)

    def folded(ap):
        # [t*128, w] dram -> [128, t, w] (lands in sbuf as chunk kc at cols kc*w)
        return ap[:].rearrange("(t p) n -> p t n", p=P)

    def unfold(sb_tile, t):
        # [128, t*w] sbuf tile viewed [128, t, w] to match folded(dram)
        return sb_tile[:].rearrange("p (t n) -> p t n", t=t)

    with tile.TileContext(nc) as tc:
        with (
            tc.tile_pool(name="sb", bufs=1) as sb,
            tc.tile_pool(name="psA", bufs=4, space="PSUM") as psA,
            tc.tile_pool(name="psB", bufs=2, space="PSUM") as psB,
            tc.tile_pool(name="psC", bufs=2, space="PSUM") as psC,
        ):
            def sbt(tag, shape, dtype=BF):
                return sb.tile(shape, dtype, name=tag, tag=tag)

            def cpst(w=512):
                return psC.tile([P, w], F32, name="mm", tag="mm")

            # ---- coalesced input DMAs, first-use order; the two big
            # critical bundles go first on the sync HWDGE queue, the small
            # consts ride the scalar HWDGE queue in parallel ----
            # q/k bundles are split: the gathered data + tile-0 weight
            # columns stream first so pair 0's projections start ~4us
            # earlier; the remaining weight tiles follow wv.
            HEADW = NSEL + P
            wqxq = sbt("wqxq", [P, CT * WQX])
            wkxk = sbt("wkxk", [P, CT * WQX])
            nc.sync.dma_start(out=unfold(wqxq, CT)[:, :, 0:HEADW],
                              in_=folded(wqxq_e)[:, :, 0:HEADW])
            nc.sync.dma_start(out=unfold(wkxk, CT)[:, :, 0:HEADW],
                              in_=folded(wkxk_e)[:, :, 0:HEADW])
            consts = sbt("consts", [P, NC_], F32)
            nc.scalar.dma_start(out=consts[:], in_=consts_e[:])
            rows2 = sbt("rows2", [1, 2 * C])
            nc.scalar.dma_start(out=rows2[:], in_=rows2_e[:])
            wv = sbt("wv", [P, CT * C])
            nc.sync.dma_start(out=unfold(wv, CT), in_=folded(wv_e))
            nc.sync.dma_start(out=unfold(wqxq, CT)[:, :, HEADW:WQX],
                              in_=folded(wqxq_e)[:, :, HEADW:WQX])
            nc.sync.dma_start(out=unfold(wkxk, CT)[:, :, HEADW:WQX],
                              in_=folded(wkxk_e)[:, :, HEADW:WQX])
            x = sbt("x", [P, CT * N])
            nc.sync.dma_start(out=unfold(x, CT), in_=folded(x_e))
            wpwv = sbt("wpwv", [P, CT * C])
            nc.sync.dma_start(out=unfold(wpwv, CT), in_=folded(wpwv_e))
            wp = sbt("wp", [P, CT * C])
            nc.sync.dma_start(out=unfold(wp, CT), in_=folded(wp_e))

            bq = consts[:, 0:CT]
            bk = consts[:, CT:2 * CT]
            bp2 = consts[:, 2 * CT:3 * CT]
            kb = consts[:, 3 * CT:3 * CT + MT]
            iqc = consts[:, 3 * CT + MT:3 * CT + 2 * MT]
            bvrow = rows2[0:1, 0:C]
            wpbv = rows2[0:1, C:2 * C]

            def xq_c(kc):
                return wqxq[:, kc * WQX:kc * WQX + NSEL]

            def wq_c(kc):
                return wqxq[:, kc * WQX + NSEL:(kc + 1) * WQX]

            def xk_c(kc):
                return wkxk[:, kc * WQX:kc * WQX + NSEL]

            def wk_c(kc):
                return wkxk[:, kc * WQX + NSEL:(kc + 1) * WQX]

            def wv_c(kc):
                return wv[:, kc * C:(kc + 1) * C]

            def wpwv_c(kc):
                return wpwv[:, kc * C:(kc + 1) * C]

            def wp_c(kc):
                return wp[:, kc * C:(kc + 1) * C]

            ones1 = sbt("ones1", [1, P])
            nc.vector.memset(ones1[:], 1.0)

            # dummy activation with no data deps: pulls the ACT_TABLE_LOAD
            # (~1.3us) to t=0 instead of serializing before the first real exp
            warm = sbt("warm", [1, 1], F32)
            nc.vector.memset(warm[:], 1.0)
            nc.scalar.activation(warm[:], warm[:], mybir.ActivationFunctionType.Exp)

            # dummy matmuls while the input DMAs land: sustained PE activity
            # flips the HAM clock gate to 2.4 GHz before real work
            wmm = sbt("wmm", [P, 512])
            nc.vector.memset(wmm[:], 0.0)
            for _ in range(7):
                wps = psC.tile([P, 512], F32, name="wps", tag="mm")
                nc.tensor.matmul(wps[:], wmm[:, :P], wmm[:], start=True, stop=True)

            # ---- scatter matrix E[j, n] = (iq[j] == n), built from a
            # GPSIMD iota row + per-partition DVE compares in DVE's idle
            # early window (saves a 768KB DMA); padded j rows use iq = -1
            # so they never match.
            # The sign of the correction is folded into the at eviction
            # (scale=-1), so E holds +1.
            emat = sbt("emat", [P, JT * N])
            it128 = sbt("iota", [P, N], F32)
            nc.gpsimd.iota(it128[:], pattern=[[1, N]], channel_multiplier=0,
                           allow_small_or_imprecise_dtypes=True)
            for jt in range(JT):
                nc.vector.tensor_scalar(
                    emat[:, jt * N:(jt + 1) * N], it128[:],
                    iqc[:, jt:jt + 1], None, op0=mybir.AluOpType.is_equal,
                )

            # ---- q/k projections: [C, NSEL] bf16; tile t+1 is emitted
            # after pair t's attention (weights for tiles 1-3 stream late)
            q_sb = sbt("q", [P, CT * NSEL])
            k_sb = sbt("k", [P, CT * NSEL])

            def emit_proj(t):
                # q and k chains interleave so each adjacent PE instruction
                # targets a different psum bank (accumulating back-to-back
                # into one bank serializes at ~2x the streaming cost)
                psq, psk = cpst(NSEL), cpst(NSEL)
                for kc in range(CT):
                    nc.tensor.matmul(
                        psq[:], wq_c(kc)[:, t * P:(t + 1) * P], xq_c(kc),
                        start=(kc == 0), stop=(kc == CT - 1),
                    )
                    nc.tensor.matmul(
                        psk[:], wk_c(kc)[:, t * P:(t + 1) * P], xk_c(kc),
                        start=(kc == 0), stop=(kc == CT - 1),
                    )
                nc.vector.tensor_scalar_add(
                    q_sb[:, t * NSEL:(t + 1) * NSEL], psq[:], bq[:, t:t + 1],
                )
                nc.vector.tensor_scalar_add(
                    k_sb[:, t * NSEL:(t + 1) * NSEL], psk[:], bk[:, t:t + 1],
                )

            emit_proj(0)

            # ---- vT_sel[m, 65h + d] = v_sel[64h + d, m]; column 65h+64 = 1.0
            # (ones column makes the PV matmul also produce Z = sum_m S[m, j])
            # Emitted inside pair 0's QK stream: pair 0 has no other PE
            # filler while ACT works through its exps.
            vt_sb = [sbt(f"vt{mt}", [P, HEADS * (HD + 1)]) for mt in range(MT)]

            def emit_vt(*mts):
                ps = {}
                for mt in mts:
                    ps[mt] = cpst(C)
                for kc in range(CT):
                    for mt in mts:
                        mo, mw = _chunks(NSEL)[mt]
                        nc.tensor.matmul(
                            ps[mt][0:mw, :],
                            xk_c(kc)[:, mo:mo + mw],
                            wv_c(kc),
                            start=(kc == 0), stop=(zbias and kc == CT - 1),
                        )
                for mt in mts:
                    mo, mw = _chunks(NSEL)[mt]
                    psm = ps[mt]
                    if not zbias:
                        nc.tensor.matmul(psm[0:mw, :], ones1[:, 0:mw], bvrow,
                                         start=False, stop=True)
                    vt_view = vt_sb[mt][:].rearrange("p (h d) -> p h d", d=HD + 1)
                    nc.vector.tensor_copy(
                        vt_view[0:mw, :, 0:HD],
                        psm[0:mw, :].rearrange("p (h d) -> p h d", d=HD),
                    )
                    nc.vector.memset(vt_view[0:mw, :, HD:HD + 1], 1.0)

            # ---- WpWv@x main-pass groups (PE filler between pairs) ----
            y_sb = sbt("y", [P, CT * N])
            ymain = [(co, nch) for co in range(CT) for nch in range(NCH)]

            def emit_ymain2(g0, g1):
                # two groups interleaved across two psum banks
                gs = [g for g in (g0, g1) if g is not None and g < len(ymain)]
                ps = [psA.tile([P, 512], F32, name="ym", tag="qk") for _ in gs]
                for kc in range(CT):
                    for gi, psm in zip(gs, ps):
                        co, nch = ymain[gi]
                        nc.tensor.matmul(
                            psm[:],
                            wpwv_c(kc)[:, co * P:(co + 1) * P],
                            x[:, kc * N + nch * 512:kc * N + (nch + 1) * 512],
                            start=(kc == 0), stop=(kc == CT - 1),
                        )
                for gi, psm in zip(gs, ps):
                    co, nch = ymain[gi]
                    seg = y_sb[:, co * N + nch * 512:co * N + (nch + 1) * 512]
                    nc.vector.tensor_scalar_add(seg, psm[:], bp2[:, co:co + 1])

            # ---- attention (selected keys m in partitions, queries j free) ----
            # S[m, j] = exp(scale * k_m . q_j + kbias[m]), bf16.  Per pair:
            # Z sits in po row 64 (vt ones column); two DVE row-copies pull it
            # to SBUF, a ones-matmul broadcasts it across the head partitions,
            # reciprocal_approx_fast (DVE, ~18 bits) gives alpha with no ACT
            # work, and on_neg = (alpha * -1) * po is fused in per-half STTs
            # reading the PV psum directly.
            po = [None] * HEADS
            on_sb = [sbt(f"on{t}", [P, NSEL]) for t in range(CT)]

            def emit_a2_head(jt, jw, jo, psm, upto):
                # A2 psum partial: optional Wp@bv bias + on-parts for pairs
                # < upto.  (No WpWv@xq term: the host zeroes the selected
                # query columns of x, so the main pass contributes nothing
                # at the scatter targets.)
                first = True
                if not zbias:
                    nc.tensor.matmul(psm[0:jw, :], ones1[:, 0:jw], wpbv,
                                     start=True, stop=False)
                    first = False
                for kc in range(upto):
                    nc.tensor.matmul(
                        psm[0:jw, :], on_sb[kc][:, jo:jo + jw], wp_c(kc),
                        start=first and kc == 0, stop=False,
                    )

            at_sb = [sbt(f"at{j}", [P, C]) for j in range(JT)]
            a2ps = [None] * JT
            ynext = 0
            for t in range(CT):  # head pair (2t, 2t+1)
                for half in range(2):
                    h = 2 * t + half
                    po[h] = psB.tile([HD + 1, NSEL], F32, name="pv", tag="pv")
                s_pairs = []
                for mj, (mo, mw) in enumerate(_chunks(NSEL)):
                    # the two QK matmuls run CONCURRENTLY on the PE via
                    # tile_position row-tiling (contraction is only 64 wide)
                    qkps = []
                    for half in range(2):
                        psm = psA.tile([P, NSEL], F32, name="qk", tag="qk")
                        nc.tensor.matmul(
                            psm[0:mw, :],
                            k_sb[half * HD:(half + 1) * HD,
                                 t * NSEL + mo:t * NSEL + mo + mw],
                            q_sb[half * HD:(half + 1) * HD,
                                 t * NSEL:(t + 1) * NSEL],
                            start=True, stop=True,
                            tile_position=(half * HD, 0),
                        )
                        qkps.append(psm)
                    s_pair = sbt(f"s{t}_{mj}", [P, 2 * NSEL])
                    s_pairs.append(s_pair)
                    for half in range(2):
                        nc.scalar.activation(
                            s_pair[0:mw, half * NSEL:(half + 1) * NSEL],
                            qkps[half][0:mw, :],
                            mybir.ActivationFunctionType.Exp,
                            bias=kb[0:mw, mj:mj + 1],
                        )
                    if t == 0:
                        # pair 0 has no projections/filler to absorb the
                        # exp latency; vt construction fills the PE instead
                        if mj == 1:
                            emit_vt(0, 1)
                        elif mj == 2:
                            emit_vt(2)
                    else:
                        if t == CT - 1 and mj == 0 and ynext < len(ymain):
                            # the last pair has no downstream projections to
                            # absorb its exp latency; spend filler here
                            emit_ymain2(ynext, ynext + 1)
                            ynext += 2
                        for half in range(2):
                            h = 2 * t + half
                            nc.tensor.matmul(
                                po[h][:],
                                vt_sb[mj][0:mw, h * (HD + 1):(h + 1) * (HD + 1)],
                                s_pair[0:mw, half * NSEL:(half + 1) * NSEL],
                                start=(mj == 0), stop=(mj == MT - 1),
                            )
                if t == 0:
                    for mj, (mo, mw) in enumerate(_chunks(NSEL)):
                        for half in range(2):
                            h = 2 * t + half
                            nc.tensor.matmul(
                                po[h][:],
                                vt_sb[mj][0:mw, h * (HD + 1):(h + 1) * (HD + 1)],
                                s_pairs[mj][0:mw, half * NSEL:(half + 1) * NSEL],
                                start=(mj == 0), stop=(mj == MT - 1),
                            )
                # per-pair 1/Z chain (overlaps the next pair's attention).
                # The next tile's projections / filler / A2 heads sit between
                # PV and the zbc broadcast in the PE stream so the PE is
                # never waiting on the DVE-side z2 row copies.
                z2 = sbt(f"z{t}", [1, 2 * NSEL])
                for half in range(2):
                    h = 2 * t + half
                    zseg = z2[0:1, half * NSEL:(half + 1) * NSEL]
                    if t == CT - 1 and half == 0:
                        # tail-critical: run the two row copies concurrently
                        # on ACT and DVE (ACT's exps are all done by now)
                        nc.scalar.activation(
                            zseg, po[h][HD:HD + 1, :],
                            mybir.ActivationFunctionType.Identity,
                        )
                    else:
                        nc.vector.tensor_copy(zseg, po[h][HD:HD + 1, :])
                if t + 1 < CT:
                    emit_proj(t + 1)
                if t in (1, 2):
                    emit_ymain2(ynext, ynext + 1)
                    ynext += 2
                if t == CT - 1:
                    # A2 heads for j-tiles 0/1 (pairs 0-2 on-parts) cover the
                    # z2 -> zbc -> recip -> STT latency of the last pair;
                    # the two chains interleave across psum banks
                    for jt in (0, 1):
                        a2ps[jt] = cpst(C)
                    first = [True, True]
                    if not zbias:
                        for jt in (0, 1):
                            jo, jw = _chunks(NSEL)[jt]
                            nc.tensor.matmul(
                                a2ps[jt][0:jw, :], ones1[:, 0:jw], wpbv,
                                start=True, stop=False)
                            first[jt] = False
                    for kc in range(CT - 1):
                        for jt in (0, 1):
                            jo, jw = _chunks(NSEL)[jt]
                            nc.tensor.matmul(
                                a2ps[jt][0:jw, :],
                                on_sb[kc][:, jo:jo + jw], wp_c(kc),
                                start=first[jt] and kc == 0, stop=False,
                            )
                zbc = psA.tile([P, NSEL], F32, name="zbc", tag="qk")
                for half in range(2):
                    nc.tensor.matmul(
                        zbc[half * HD:(half + 1) * HD, :],
                        ones1[:, 0:HD],
                        z2[0:1, half * NSEL:(half + 1) * NSEL],
                        start=True, stop=True,
                    )
                if t == CT - 1 and ynext < len(ymain):
                    emit_ymain2(ynext, ynext + 1)  # covers recip+STT latency
                    ynext += 2
                alpbc = sbt(f"alp{t}", [P, NSEL], F32)
                nc.vector.reciprocal_approx_fast(out=alpbc[:], in_=zbc[:])
                for half in range(2):
                    h = 2 * t + half
                    nc.vector.scalar_tensor_tensor(
                        on_sb[t][half * HD:(half + 1) * HD, :],
                        alpbc[half * HD:(half + 1) * HD, :], -1.0,
                        po[h][0:HD, :],
                        op0=mybir.AluOpType.mult, op1=mybir.AluOpType.mult,
                    )

            # ---- A2^T[j, :] = (Wp@(-on) + Wp@bv)^T, negated at eviction --
            # j-tiles 0/1 only need the pair-3 on chunk; tile 2 runs in full
            # on a psB bank, its chain interleaved with the 0/1 finishers.
            # Evictions ride ScalarE (idle after the last exp).
            jo2, jw2 = _chunks(NSEL)[2]
            a2ps[2] = psB.tile([P, C], F32, name="pv", tag="pv")
            first2 = True
            if not zbias:
                nc.tensor.matmul(a2ps[2][0:jw2, :], ones1[:, 0:jw2], wpbv,
                                 start=True, stop=False)
                first2 = False
            for kc in range(CT):
                nc.tensor.matmul(
                    a2ps[2][0:jw2, :], on_sb[kc][:, jo2:jo2 + jw2], wp_c(kc),
                    start=first2 and kc == 0, stop=(kc == CT - 1),
                )
                if kc < 2:
                    jo, jw = _chunks(NSEL)[kc]
                    nc.tensor.matmul(
                        a2ps[kc][0:jw, :], on_sb[CT - 1][:, jo:jo + jw],
                        wp_c(CT - 1), start=False, stop=True,
                    )
            for jt, (jo, jw) in enumerate(_chunks(NSEL)):
                if jt == 0:
                    # DVE and ACT evict concurrently; YE only gates on at0
                    nc.vector.tensor_scalar_mul(
                        at_sb[jt][0:jw, :], a2ps[jt][0:jw, :], -1.0)
                else:
                    nc.scalar.activation(
                        at_sb[jt][0:jw, :], a2ps[jt][0:jw, :],
                        mybir.ActivationFunctionType.Identity, scale=-1.0,
                    )

            # ---- y += at^T @ E (column scatter of the correction) ----
            # groups run in interleaved pairs across two psum banks
            for co in range(CT):
                ps = [psB.tile([P, 512], F32, name="ye", tag="pv")
                      for _ in range(NCH)]
                for jt, (jo, jw) in enumerate(_chunks(NSEL)):
                    for nch in range(NCH):
                        nc.tensor.matmul(
                            ps[nch][:],
                            at_sb[jt][0:jw, co * P:(co + 1) * P],
                            emat[0:jw, jt * N + nch * 512:jt * N + (nch + 1) * 512],
                            start=(jt == 0), stop=(jt == JT - 1),
                        )
                for nch in range(NCH):
                    seg = y_sb[:, co * N + nch * 512:co * N + (nch + 1) * 512]
                    nc.vector.tensor_tensor(
                        seg, seg, ps[nch][:], op=mybir.AluOpType.add,
                    )
                    nc.scalar.dma_start(
                        out=y_e[:]
                        .rearrange("(t p) n -> t p n", p=P)[co]
                        [:, nch * 512:(nch + 1) * 512],
                        in_=seg,
                    )

    # The greedy ACT-table-load pass alternates between exp-only and ln-only
    # table sets for our Exp/Ln/Identity/Copy mix, inserting ~9 ACT_TABLE_LOADs
    # (~1.3us each).  natural_log_exp_and_others contains all four functions;
    # make it the only candidate (list positions must stay aligned with
    # act_info.json indices, so empty the competitors instead of removing).
    import concourse.bacc as bacc_mod

    WANT = "natural_log_exp_and_others"
    orig_tables = bacc_mod.get_activation_tables

    def one_set_tables(arch):
        tabs = orig_tables(arch)
        ours = {
            mybir.ActivationFunctionType.Exp,
            mybir.ActivationFunctionType.Ln,
            mybir.ActivationFunctionType.Identity,
            mybir.ActivationFunctionType.Copy,
        }
        return {
            name: (fns if name == WANT else fns - ours)
            for name, fns in tabs.items()
        }

    bacc_mod.get_activation_tables = one_set_tables
    try:
        nc.compile()
    finally:
        bacc_mod.get_activation_tables = orig_tables
    return nc


def _get_program(NSEL, zbias):
    key = (NSEL, zbias)
    if key not in _PROGRAM_CACHE:
        _PROGRAM_CACHE[key] = _build_program(NSEL, zbias)
    return _PROGRAM_CACHE[key]


def _sel_masks(x, u, ws, bs):
    """Bit-exact replica of the reference's gumbel argmax mask (fp32, CPU jax)."""
    import jax
    import jax.numpy as jnp

    cpu = jax.devices("cpu")[0]
    with jax.default_device(cpu):
        xj = jax.device_put(jnp.asarray(x, jnp.float32), cpu)
        uj = jax.device_put(jnp.asarray(u, jnp.float32), cpu)
        wj = jax.device_put(jnp.asarray(ws, jnp.float32), cpu)
        bj = jax.device_put(jnp.asarray(bs, jnp.float32), cpu)
        logits = jnp.einsum("bchw,oc->bohw", xj, wj) + bj[None, :, None, None]
        g = -jnp.log(-jnp.log(uj + EPS) + EPS)
        m = jnp.argmax(logits + g, axis=1) == 0
        return np.asarray(m).reshape(x.shape[0], N)


def _col_layout(vec, nt):
    """[nt*128] -> [128, nt] with column t = vec[128t:128(t+1)]."""
    return np.ascontiguousarray(vec.reshape(nt, P).T)


def kernel(x, u_q, u_k, wq_s, bq_s, wk_s, bk_s, Wq, bq, Wk, bk, Wv, bv, Wp, bp):
    global LAST_RESULT
    x = np.asarray(x, np.float32)
    u_q, u_k = np.asarray(u_q, np.float32), np.asarray(u_k, np.float32)

    mq = _sel_masks(x, u_q, np.asarray(wq_s), np.asarray(bq_s))
    mk = _sel_masks(x, u_k, np.asarray(wk_s), np.asarray(bk_s))

    idx_q = [np.nonzero(mq[b])[0] for b in range(B)]
    idx_k = [np.nonzero(mk[b])[0] for b in range(B)]
    max_cnt = max(max(len(i) for i in idx_q), max(len(i) for i in idx_k))
    NSEL = NSEL_DEFAULT
    while NSEL < max_cnt:
        NSEL += 32
    MT = len(_chunks(NSEL))
    JTP = MT * P

    Wq32, Wk32 = np.asarray(Wq, np.float32), np.asarray(Wk, np.float32)
    Wv32, Wp32 = np.asarray(Wv, np.float32), np.asarray(Wp, np.float32)
    bq32 = np.asarray(bq, np.float32)
    bk32 = np.asarray(bk, np.float32) * SCALE
    bv32, bp32 = np.asarray(bv, np.float32), np.asarray(bp, np.float32)

    WpWv32 = Wp32 @ Wv32
    wpbv32 = Wp32 @ bv32
    bp2 = bp32 + wpbv32

    wqT = np.ascontiguousarray(Wq32.T)
    wkT = np.ascontiguousarray(Wk32.T) * SCALE   # fold the softmax scale into k
    wvT = np.ascontiguousarray(Wv32.T).astype(BF16)
    wpT = np.ascontiguousarray(Wp32.T).astype(BF16)
    wpwvT = np.ascontiguousarray(WpWv32.T).astype(BF16)
    rows2 = np.concatenate([bv32, wpbv32]).reshape(1, 2 * C).astype(BF16)

    # consts [P, 8 + CT + MT]
    kbias_cols = MT

    xf = x.reshape(B, C, N)
    in_maps = []
    for b in range(B):
        iq, ik = idx_q[b], idx_k[b]
        cq, ck = len(iq), len(ik)
        iq_pad = np.pad(iq, (0, NSEL - cq))
        ik_pad = np.pad(ik, (0, NSEL - ck))

        kbias = np.zeros(MT * P, np.float32)
        kbias[ck:NSEL] = NEG
        kbias[NSEL:] = NEG
        iqf = np.full(MT * P, -1.0, np.float32)
        iqf[:cq] = iq.astype(np.float32)
        consts = np.concatenate([
            _col_layout(bq32, CT), _col_layout(bk32, CT),
            _col_layout(bp2, CT),
            _col_layout(kbias, MT)[:, :kbias_cols],
            _col_layout(iqf, MT),
        ], axis=1).astype(np.float32)
        consts = np.ascontiguousarray(consts)

        xb = xf[b].copy()
        xb[:, iq] = 0.0            # main pass must not contribute at scatter targets
        xb = xb.astype(BF16)
        xqg = np.ascontiguousarray(xf[b][:, iq_pad]).astype(BF16)
        xkg = np.ascontiguousarray(xf[b][:, ik_pad]).astype(BF16)
        wqxq = np.ascontiguousarray(
            np.concatenate([xf[b][:, iq_pad], wqT], axis=1)).astype(BF16)
        wkxk = np.ascontiguousarray(
            np.concatenate([xf[b][:, ik_pad], wkT], axis=1)).astype(BF16)
        del xqg, xkg

        in_maps.append({
            "consts": consts,
            "rows2": rows2,
            "wqxq": wqxq,
            "wkxk": wkxk,
            "wvT": wvT,
            "xbf": xb,
            "wpwvT": wpwvT,
            "wpT": wpT,
        })

    zbias = not (bq32.any() or bk32.any() or bv32.any() or bp32.any())
    nc = _get_program(NSEL, zbias)
    res = run_bass_kernel_spmd(nc, in_maps, list(range(B)), trace=TRACE)
    LAST_RESULT = res

    y = np.stack([np.asarray(res.results[b]["y"], np.float32) for b in range(B)])
    return y.reshape(B, C, H, W)
